# revision 55
# baseline (speedup 1.0000x reference)
"""Trainium2 Bass kernel: causal self-attention with RoPE (B=2, T=2048, D=2048, H=16).

Sharding: 8 cores = 2-way data parallel over batch x 4-way tensor parallel over
heads.  Core c = 4*b + g computes batch b, heads 4g..4g+3, and produces a
partial output y_partial = attn_out[:, heads_g] @ w_proj[:, heads_g].T which the
host sums over g.

Per-core pipeline (v2 — tuned against the TimelineSim cost model):
  - qkv projection in fp32r (full-rate at free>=256), weights DMA'd in 512-col
    chunks interleaved with the x stream so the first matmul starts ~4us in.
  - q/k psums staged to SBUF by ScalarE copies (frees PSUM banks immediately;
    6-slot mm psum pool), RoPE on DVE reads the stage off the critical path.
  - exp on ScalarE writes p~ directly as bf16; PV matmul (free size 129) runs
    in bf16 at 1 cycle/row (fp32r would be 4x slower below 256 free).
  - o -> PE transpose in bf16 -> oT bf16 feeds the output projection (bf16
    weights) interleaved into head 3's attention, one t-range per i-block.
  - RoPE cos / sign-folded sin, causal 0/1 big-mask, identity are
    host-precomputed; consts stream in per-t-block chunks after the first
    weight chunks.
"""

import sys

import numpy as np
import ml_dtypes

for _p in ("/opt/trn_rl_repo", "/root/.axon_site/_ro/trn_rl_repo"):
    if _p not in sys.path:
        sys.path.append(_p)

import concourse.bass as bass
import concourse.bacc as bacc
import concourse.tile as tile
from concourse import mybir
from concourse.bass_utils import run_bass_kernel_spmd

F32 = mybir.dt.float32
BF16 = mybir.dt.bfloat16
AF = mybir.ActivationFunctionType

B, T, D, H = 2, 2048, 2048, 16
HPC = H // 4  # heads per core (4-way head TP)
DH = D // H   # 128
SCALE = float(DH) ** -0.5

TB = 512      # qkv-projection t-block (psum free width)
SB = 512      # attention i-block (score free width)


def build_nc(t=T, mm_dt="float32r", pv_dt="bfloat16"):
    """Build the SPMD per-core program.  `t` is the sequence length (smaller
    values are used for simulator validation)."""
    NT = t // 128    # token tiles
    TBE = min(TB, t)
    NTB = t // TBE   # qkv t-blocks
    sb = min(SB, t)
    NSB = t // sb    # attention i-blocks
    NIC = sb // 128  # i-chunks per i-block
    ND = D // 128    # contraction d-tiles
    C0 = sb - 128    # base column of the causal big-mask

    MDT = mybir.dt.float32r if (mm_dt == "float32r") else F32
    BDT = BF16 if (pv_dt == "bfloat16") else F32

    nc = bacc.Bacc("TRN2", target_bir_lowering=False, debug=False)

    xT = nc.dram_tensor("xT", [D, t], MDT, kind="ExternalInput").ap()
    wqh = nc.dram_tensor("wqh", [128, HPC * D], MDT, kind="ExternalInput").ap()
    wkh = nc.dram_tensor("wkh", [128, HPC * D], MDT, kind="ExternalInput").ap()
    wvh = nc.dram_tensor("wvh", [128, (HPC // 2) * 2 * D], MDT, kind="ExternalInput").ap()
    wpT = nc.dram_tensor("wpT", [HPC * DH, D], BDT, kind="ExternalInput").ap()
    cosT = nc.dram_tensor("cosT", [DH, t], BDT, kind="ExternalInput").ap()
    sinTm = nc.dram_tensor("sinTm", [DH, t], BDT, kind="ExternalInput").ap()
    bmask = nc.dram_tensor("bmask", [128, 128], BDT, kind="ExternalInput").ap()
    ident = nc.dram_tensor("ident", [128, 128], BDT, kind="ExternalInput").ap()
    rotm = nc.dram_tensor("rotm", [128, 128], BDT, kind="ExternalInput").ap()
    y = nc.dram_tensor("y", [t, D], F32, kind="ExternalOutput").ap()

    with tile.TileContext(nc) as tc:
        with (
            tc.tile_pool(name="consts", bufs=1) as cpool,
            tc.tile_pool(name="oTp", bufs=1) as opool,
            tc.tile_pool(name="qkp", bufs=2) as qkpool,
            tc.tile_pool(name="xtp", bufs=6) as xtp,
            tc.tile_pool(name="wqkp", bufs=1) as wqkp,
            tc.tile_pool(name="wvp", bufs=1) as wvp,
            tc.tile_pool(name="wpj", bufs=1) as wpj,
            tc.tile_pool(name="vep", bufs=2) as vep,
            tc.tile_pool(name="ptp", bufs=2 * NT + 2) as ptp,
            tc.tile_pool(name="tmpp", bufs=4) as tmpp,
            tc.tile_pool(name="smallp", bufs=6) as smallp,
            tc.tile_pool(name="ysp", bufs=4) as ysp,
            tc.tile_pool(name="ps_mm", bufs=6, space="PSUM") as ps_mm,
            tc.tile_pool(name="ps_sm", bufs=2, space="PSUM") as ps_sm,
        ):
            # const tiles allocated up front; DMAs issued later (chunked) so
            # the weight/x stream owns the head of the DMA queue.
            cos_sb = cpool.tile([DH, t], BDT, tag="cos")
            sin_sb = cpool.tile([DH, t], BDT, tag="sin")
            bm_sb = cpool.tile([128, 128], BDT, tag="bm")
            id_sb = cpool.tile([128, 128], BDT, tag="id")
            rot_sb = cpool.tile([128, 128], BDT, tag="rot")
            oT_sb = [
                opool.tile([DH, t], BDT, tag=f"oT{h}", name=f"oT{h}")
                for h in range(HPC)
            ]
            wp_sb = [
                wpj.tile([128, D], BDT, tag=f"wp{h}", name=f"wp{h}")
                for h in range(HPC)
            ]

            def rope_ip(qk, tb, name):
                """In-place RoPE on qk[:, tb-block] (holds the raw projection,
                parked there by a ScalarE psum copy).  The half-rotation (a
                cross-partition shuffle the DVE can't address) runs on the PE
                as a signed permutation matmul; the remaining DVE ops are all
                partition-aligned bf16 (2x mode)."""
                t0, t1_ = TBE * tb, TBE * (tb + 1)
                blk = qk[:, t0:t1_]
                rps = ps_mm.tile([128, TBE], F32, tag="mm", name=f"rot_{name}")
                nc.tensor.matmul(rps[:], rot_sb[:], blk, start=True, stop=True)
                r1 = tmpp.tile([128, TBE], BDT, tag="r1", name=f"r1_{name}")
                nc.vector.tensor_mul(r1[:], blk, cos_sb[:, t0:t1_])
                r2 = tmpp.tile([128, TBE], BDT, tag="r2", name=f"r2_{name}")
                nc.vector.tensor_mul(r2[:], rps[:], sin_sb[:, t0:t1_])
                nc.vector.tensor_add(blk, r1[:], r2[:])

            def emit_proj(tt, on_act=False):
                """y[128*tt:128*(tt+1), :] = sum_h oT_h[:, tt].T @ wp_h.
                on_act routes the psum drain to ScalarE (used for the blocks
                issued ahead of B chunks, whose DVE normalizes must not queue
                behind these copies)."""
                for db in range(D // 512):
                    ps = ps_mm.tile([128, 512], F32, tag="mm", name=f"psy{tt}_{db}")
                    for h in range(HPC):
                        nc.tensor.matmul(
                            ps[:],
                            oT_sb[h][:, 128 * tt:128 * (tt + 1)],
                            wp_sb[h][:, 512 * db:512 * (db + 1)],
                            start=(h == 0), stop=(h == HPC - 1))
                    yst = ysp.tile([128, 512], F32, tag="yst", name=f"yst{tt}_{db}")
                    if on_act:
                        nc.scalar.copy(yst[:], ps[:])
                    else:
                        nc.vector.tensor_copy(yst[:], ps[:])
                    nc.sync.dma_start(
                        y[128 * tt:128 * (tt + 1), 512 * db:512 * (db + 1)], yst[:])

            pair_bufs = {}
            pend_ropes = []

            def qkv_pair_steps(p2):
                """Generator: pair-p2 weight DMAs + qkv projection; yields
                once per t-block so attention chunks of the previous pair can
                interleave into the PE stream."""
                h = 2 * p2
                q_sbs = [None, None]
                k_sbs = [None, None]
                vext = [None, None]
                wq0 = wqkp.tile([128, D], MDT, tag="wq0", name=f"wq0_{p2}")
                wk0 = wqkp.tile([128, D], MDT, tag="wk0", name=f"wk0_{p2}")
                wq1 = wqkp.tile([128, D], MDT, tag="wq1", name=f"wq1_{p2}")
                wk1 = wqkp.tile([128, D], MDT, tag="wk1", name=f"wk1_{p2}")
                wv_sb = wvp.tile([128, 2 * D], MDT, tag="wv", name=f"wv{p2}")
                vext[0] = vep.tile([128, NT * 129], BDT, tag="ve0", name=f"ve0_{p2}")
                vext[1] = vep.tile([128, NT * 129], BDT, tag="ve1", name=f"ve1_{p2}")
                nc.vector.memset(vext[0][:], 1.0)
                nc.vector.memset(vext[1][:], 1.0)
                q_sbs[0] = qkpool.tile([DH, t], BDT, tag="q0", name=f"q0_{p2}")
                k_sbs[0] = qkpool.tile([DH, t], BDT, tag="k0", name=f"k0_{p2}")
                q_sbs[1] = qkpool.tile([DH, t], BDT, tag="q1", name=f"q1_{p2}")
                k_sbs[1] = qkpool.tile([DH, t], BDT, tag="k1", name=f"k1_{p2}")
                pair_bufs[p2] = {"q": q_sbs, "k": k_sbs, "ve": vext}

                if p2 == 1:
                    # prefetch the projection weights during pair-1 qkv
                    for hh in range(HPC):
                        nc.sync.dma_start(wp_sb[hh][:], wpT[128 * hh:128 * (hh + 1), :])

                # ---- qkv projection for the pair (one xT pass) ----
                ttpb = TBE // 128
                nvp = (ttpb + 1) // 2
                for tb in range(NTB):
                    t0, t1_ = TBE * tb, TBE * (tb + 1)
                    ps_q0 = ps_mm.tile([128, TBE], F32, tag="mm", name=f"psq0_{p2}_{tb}")
                    ps_k0 = ps_mm.tile([128, TBE], F32, tag="mm", name=f"psk0_{p2}_{tb}")
                    ps_q1 = ps_mm.tile([128, TBE], F32, tag="mm", name=f"psq1_{p2}_{tb}")
                    ps_k1 = ps_mm.tile([128, TBE], F32, tag="mm", name=f"psk1_{p2}_{tb}")
                    ps_vs = [
                        ps_sm.tile([128, 512], F32, tag="sm", name=f"psv{p2}_{tb}_{i}")
                        for i in range(nvp)
                    ]
                    for di in range(ND):
                        d0, d1 = 128 * di, 128 * (di + 1)
                        # weight chunks interleaved with the x stream so the
                        # first matmuls start after ~0.5MB of DMA
                        wq_chunks = []
                        if tb == 0 and di % 4 == 0:
                            wq_chunks = [(512 * (di // 4), 512 * (di // 4 + 1))]
                        for c0_, c1_ in wq_chunks:
                            nc.sync.dma_start(wq0[:, c0_:c1_], wqh[:, D * h + c0_:D * h + c1_])
                        xt_t = xtp.tile([128, TBE], MDT, tag="xt", name=f"xt{p2}_{tb}_{di}")
                        nc.sync.dma_start(xt_t[:], xT[d0:d1, t0:t1_])
                        for c0_, c1_ in wq_chunks:
                            nc.sync.dma_start(wk0[:, c0_:c1_], wkh[:, D * h + c0_:D * h + c1_])
                            nc.sync.dma_start(wq1[:, c0_:c1_], wqh[:, D * (h + 1) + c0_:D * (h + 1) + c1_])
                            nc.sync.dma_start(wk1[:, c0_:c1_], wkh[:, D * (h + 1) + c0_:D * (h + 1) + c1_])
                        if tb == 0 and di % 2 == 0:
                            v0, v1 = 512 * (di // 2), 512 * (di // 2 + 1)
                            nc.sync.dma_start(wv_sb[:, v0:v1], wvh[:, 2 * D * p2 + v0:2 * D * p2 + v1])
                        st, sp = di == 0, di == ND - 1
                        nc.tensor.matmul(ps_q0[:], wq0[:, d0:d1], xt_t[:], start=st, stop=sp)
                        nc.tensor.matmul(ps_k0[:], wk0[:, d0:d1], xt_t[:], start=st, stop=sp)
                        nc.tensor.matmul(ps_q1[:], wq1[:, d0:d1], xt_t[:], start=st, stop=sp)
                        nc.tensor.matmul(ps_k1[:], wk1[:, d0:d1], xt_t[:], start=st, stop=sp)
                        for tt in range(ttpb):
                            # start=True clears the whole bank: only the
                            # even half may set it; the odd half's first
                            # write lands via has_written=0 overwrite.
                            nc.tensor.matmul(
                                ps_vs[tt // 2][:, 256 * (tt % 2):256 * (tt % 2) + 256],
                                xt_t[:, 128 * tt:128 * (tt + 1)],
                                wv_sb[:, 256 * di:256 * (di + 1)],
                                start=(st and tt % 2 == 0), stop=sp,
                                skip_group_check=True)
                    if p2 == 0:
                        # stream the consts the tb's ropes need right
                        # behind its x tiles
                        nc.sync.dma_start(cos_sb[:, t0:t1_], cosT[:, t0:t1_])
                        nc.sync.dma_start(sin_sb[:, t0:t1_], sinTm[:, t0:t1_])
                        if tb == 0:
                            nc.sync.dma_start(rot_sb[:], rotm[:])
                        if tb == 1:
                            nc.sync.dma_start(bm_sb[:], bmask[:])
                            nc.sync.dma_start(id_sb[:], ident[:])
                    # park raw psums straight into the q/k tiles on
                    # ScalarE (frees the banks without waiting on DVE);
                    # RoPE rotates in place later.  At pair 0's last t-block
                    # the first attention chunks are waiting on these slots,
                    # so split the copies across ScalarE and DVE.
                    split = tb == NTB - 1
                    for i_, (ps, dst) in enumerate(((ps_q0, q_sbs[0]), (ps_k0, k_sbs[0]), (ps_q1, q_sbs[1]), (ps_k1, k_sbs[1]))):
                        if split and i_ >= 2:
                            nc.vector.tensor_copy(dst[:, t0:t1_], ps[:])
                        else:
                            nc.scalar.copy(dst[:, t0:t1_], ps[:])
                    for tt in range(ttpb):
                        gt = tb * ttpb + tt
                        o0 = 256 * (tt % 2)
                        nc.scalar.copy(
                            vext[0][:, 129 * gt:129 * gt + 128],
                            ps_vs[tt // 2][:, o0:o0 + 128])
                        nc.scalar.copy(
                            vext[1][:, 129 * gt:129 * gt + 128],
                            ps_vs[tt // 2][:, o0 + 128:o0 + 256])
                    # RoPE policy: pair 0 defers q1/k1 + the last tb's q0/k0
                    # into the attention stream (so the pair-boundary DVE
                    # queue starts with the diagonal masks).  Pair 1's tb
                    # loop is already interleaved with h0/h1 attention, so
                    # in-loop ropes hide in the t-block gaps; only the last
                    # tb's four ropes defer (consumed by h2's chunks).
                    last = tb == NTB - 1
                    if not last:
                        rope_ip(q_sbs[0], tb, f"q0_{p2}_{tb}")
                        rope_ip(k_sbs[0], tb, f"k0_{p2}_{tb}")
                        if p2 == 1:
                            rope_ip(q_sbs[1], tb, f"q1_{p2}_{tb}")
                            rope_ip(k_sbs[1], tb, f"k1_{p2}_{tb}")
                    else:
                        pend_ropes.append((q_sbs[0], tb, f"q0_{p2}_{tb}"))
                        pend_ropes.append((k_sbs[0], tb, f"k0_{p2}_{tb}"))
                        pend_ropes.append((q_sbs[1], tb, f"q1_{p2}_{tb}"))
                        pend_ropes.append((k_sbs[1], tb, f"k1_{p2}_{tb}"))
                    if p2 == 0 and not last:
                        pend_ropes.append((q_sbs[1], tb, f"q1_{p2}_{tb}"))
                        pend_ropes.append((k_sbs[1], tb, f"k1_{p2}_{tb}"))
                    yield

            def attention_steps(h, quotas, pipelined, do_proj=False):
                """Generator for head h's attention, yielding once per chunk.
                pipelined=True: A0 A1 B0 A2 B1 A3 B2 B3 (B lags A by one).
                pipelined=False: A0 B0 A1 B1 ... (B right after its A).
                quotas[i] deferred ropes are issued after the i-th B chunk."""
                par = h % 2
                bufs = pair_bufs[h // 2]
                q_sb, k_sb = bufs["q"][par], bufs["k"][par]
                ve = bufs["ve"][par]
                if par == 0 and pend_ropes:
                    # rotate this head's q/k first (its own ib3 needs them)
                    mine = [e for e in pend_ropes if e[0] is q_sb or e[0] is k_sb]
                    rest = [e for e in pend_ropes if not (e[0] is q_sb or e[0] is k_sb)]
                    pend_ropes[:] = mine + rest

                def stage_a(ib):
                    """scores + exp + diagonal mask for i-block ib."""
                    i0 = sb * ib
                    jt_max = (i0 + sb) // 128 - 1  # inclusive
                    pts = [None] * (jt_max + 1)
                    for jt in range(jt_max + 1):
                        s_ps = ps_mm.tile([128, sb], F32, tag="mm", name=f"s{h}_{ib}_{jt}")
                        nc.tensor.matmul(
                            s_ps[:],
                            k_sb[:, 128 * jt:128 * (jt + 1)],
                            q_sb[:, i0:i0 + sb],
                            start=True, stop=True)
                        pt_t = ptp.tile([128, sb], BDT, tag="pt", name=f"pt{h}_{ib}_{jt}")
                        nc.scalar.activation(pt_t[:], s_ps[:], AF.Exp, scale=SCALE)
                        m = jt - NIC * ib
                        if m >= 0:
                            # only the 128x128 sub-block straddling the causal
                            # diagonal needs masking: fully-masked sub-blocks
                            # are never read by the PV loop bounds in stage_b
                            pm = pt_t[:, 128 * m:128 * (m + 1)]
                            nc.vector.tensor_mul(pm, pm, bm_sb[:])
                        pts[jt] = pt_t
                    return pts

                def stage_b(ib, pts, bi):
                    """PV + normalize + transpose for i-block ib."""
                    i0 = sb * ib

                    def finish(ic, pv):
                        rc = smallp.tile([128, 1], F32, tag="rc", name=f"rc{h}_{ib}_{ic}")
                        nc.vector.reciprocal(rc[:], pv[:, 128:129])
                        o_sb = smallp.tile([128, 128], BDT, tag="o", name=f"o{h}_{ib}_{ic}")
                        nc.vector.tensor_scalar_mul(o_sb[:], pv[:, 0:128], rc[:])
                        ot_ps = ps_mm.tile([128, 128], BDT, tag="mm", name=f"otp{h}_{ib}_{ic}")
                        nc.tensor.transpose(ot_ps[:], o_sb[:], id_sb[:])
                        c0 = i0 + 128 * ic
                        nc.vector.tensor_copy(oT_sb[h][:, c0:c0 + 128], ot_ps[:])

                    prev = None
                    for ic in range(NIC):
                        last_jt = NIC * ib + ic
                        pv = ps_sm.tile([128, 129], F32, tag="sm", name=f"pv{h}_{ib}_{ic}")
                        for jt in range(last_jt + 1):
                            nc.tensor.matmul(
                                pv[:],
                                pts[jt][:, 128 * ic:128 * (ic + 1)],
                                ve[:, 129 * jt:129 * (jt + 1)],
                                start=(jt == 0), stop=(jt == last_jt))
                        if prev is not None:
                            finish(*prev)
                        prev = (ic, pv)
                    finish(*prev)
                    if do_proj:
                        # head 3's oT columns complete per i-block: fold the
                        # output projection for those t-tiles in right away
                        # (the last i-block drains on both engines so the
                        # kernel tail is DMA-limited, not copy-limited)
                        for tt in range(NIC * ib, NIC * (ib + 1)):
                            emit_proj(tt, on_act=(ib == NSB - 1 and tt % 2 == 0))
                    # deferred RoPEs enter the DVE queue after the B chunk so
                    # the masks and PV-psum-freeing normalizes stay ahead
                    consume_ropes(quotas[bi])

                if pipelined:
                    pts_prev = None
                    for ib in range(NSB):
                        pts_cur = stage_a(ib)
                        yield
                        if pts_prev is not None:
                            stage_b(ib - 1, pts_prev, ib - 1)
                            yield
                        pts_prev = pts_cur
                    stage_b(NSB - 1, pts_prev, NSB - 1)
                    yield
                else:
                    for ib in range(NSB):
                        pts = stage_a(ib)
                        yield
                        stage_b(ib, pts, ib)
                        yield

            def stepn(g, n):
                for _ in range(n):
                    next(g)

            def consume_ropes(n):
                for qk, tb_, nm_ in pend_ropes[:n]:
                    rope_ip(qk, tb_, nm_)
                del pend_ropes[:n]

            # ---- schedule -------------------------------------------
            # pair 0 qkv alone (nothing to overlap), then h0/h1 attention
            # chunks interleaved into pair 1's t-blocks (attention is
            # exp-bound on ScalarE; the qkv matmuls keep PE busy), then
            # h2/h3 attention interleaved with the output projection.
            for _ in qkv_pair_steps(0):
                pass
            for _ in attention_steps(0, [3, 3, 2, 2], True):
                pass
            for _ in attention_steps(1, [0, 0, 0, 0], True):
                pass
            for _ in qkv_pair_steps(1):
                pass
            for _ in attention_steps(2, [2, 2, 0, 0], True):
                pass
            for _ in attention_steps(3, [0, 0, 0, 0], True, do_proj=True):
                pass

    nc.compile()
    return nc


def host_consts(t=T):
    """RoPE cos/sin (f32, matching the jax reference), causal big-mask, identity."""
    inv = (1.0 / (np.float32(10000.0) ** (np.arange(0, DH, 2, dtype=np.float32) / np.float32(DH)))).astype(np.float32)
    tt = np.arange(t, dtype=np.float32)
    fr = np.outer(tt, inv).astype(np.float32)       # [t, 64]
    emb = np.concatenate([fr, fr], axis=1)          # [t, 128]
    cosT = np.ascontiguousarray(np.cos(emb).T.astype(np.float32))
    sinTm = np.ascontiguousarray(np.sin(emb).T.astype(np.float32))
    jj = np.arange(128)[:, None]
    cc = np.arange(128)[None, :]
    bmask = (cc >= jj).astype(np.float32)
    ident = np.eye(128, dtype=np.float32)
    # signed half-rotation: (rotm.T @ x)[d] = -x[d+64] for d<64, x[d-64] else
    rotm = np.zeros((128, 128), dtype=np.float32)
    for d in range(64):
        rotm[d + 64, d] = -1.0
        rotm[d, d + 64] = 1.0
    return cosT, sinTm, bmask, ident, rotm


def _warrange(w):
    """[128*nh rows, D] head-major weight slice -> [128, nh*D] sbuf-ready layout:
    block h, col di*128+c of partition p  =  w[128*h + c, 128*di + p]."""
    nh = w.shape[0] // 128
    d = w.shape[1]
    out = np.empty((128, nh * d), dtype=np.float32)
    for h in range(nh):
        a = w[128 * h:128 * (h + 1), :].T.reshape(d // 128, 128, 128)  # [di, p, c]
        out[:, d * h:d * (h + 1)] = a.transpose(1, 0, 2).reshape(128, d)
    return out


def _wvarrange(w):
    """[512 rows, D] 4-head v-weights -> [128, 2*2*D]: per pair, di-major blocks of
    [even-head 128 cols | odd-head 128 cols]."""
    d = w.shape[1]
    blocks = []
    for p2 in range(2):
        e = w[256 * p2:256 * p2 + 128, :].T.reshape(d // 128, 128, 128)
        o = w[256 * p2 + 128:256 * p2 + 256, :].T.reshape(d // 128, 128, 128)
        pair = np.concatenate([e, o], axis=2)          # [di, p, 256]
        blocks.append(pair.transpose(1, 0, 2).reshape(128, 2 * d))
    return np.concatenate(blocks, axis=1)


def shard_inputs(x, w_qkv, w_proj, t=T, pv_dt="bfloat16"):
    """Build the 8 per-core input maps."""
    bdt = ml_dtypes.bfloat16 if pv_dt == "bfloat16" else np.float32
    cosT, sinTm, bmask, ident, rotm = host_consts(t)
    cosT = cosT.astype(bdt)
    sinTm = sinTm.astype(bdt)
    bmask = bmask.astype(bdt)
    ident = ident.astype(bdt)
    rotm = rotm.astype(bdt)
    d = x.shape[2]
    in_maps = []
    for c in range(8):
        b, g = divmod(c, 4)
        s0, s1 = 512 * g, 512 * (g + 1)
        in_maps.append(dict(
            xT=np.ascontiguousarray(x[b].T),
            wqh=_warrange(w_qkv[s0:s1, :]),
            wkh=_warrange(w_qkv[d + s0:d + s1, :]),
            wvh=_wvarrange(w_qkv[2 * d + s0:2 * d + s1, :]),
            wpT=np.ascontiguousarray(w_proj[:, s0:s1].T).astype(bdt),
            cosT=cosT, sinTm=sinTm, bmask=bmask, ident=ident, rotm=rotm,
        ))
    return in_maps


_NC_CACHE = {}


def get_nc(t=T, mm_dt="float32r", pv_dt="bfloat16"):
    key = (t, mm_dt, pv_dt)
    if key not in _NC_CACHE:
        _NC_CACHE[key] = build_nc(t=t, mm_dt=mm_dt, pv_dt=pv_dt)
    return _NC_CACHE[key]


def kernel(x, w_qkv, w_proj):
    x = np.asarray(x, dtype=np.float32)
    w_qkv = np.asarray(w_qkv, dtype=np.float32)
    w_proj = np.asarray(w_proj, dtype=np.float32)
    b_, t_, d_ = x.shape
    in_maps = shard_inputs(x, w_qkv, w_proj, t=t_)
    nc = get_nc(t=t_)
    res = run_bass_kernel_spmd(nc, in_maps, list(range(8))).results
    out = np.zeros((b_, t_, d_), dtype=np.float32)
    for c in range(8):
        b, _ = divmod(c, 4)
        out[b] += res[c]["y"]
    return out


# revision 57
# speedup vs baseline: 1.0090x; 1.0090x over previous
"""Trainium2 Bass kernel: causal self-attention with RoPE (B=2, T=2048, D=2048, H=16).

Sharding: 8 cores = 2-way data parallel over batch x 4-way tensor parallel over
heads.  Core c = 4*b + g computes batch b, heads 4g..4g+3, and produces a
partial output y_partial = attn_out[:, heads_g] @ w_proj[:, heads_g].T which the
host sums over g.

Per-core pipeline (v2 — tuned against the TimelineSim cost model):
  - qkv projection in fp32r (full-rate at free>=256), weights DMA'd in 512-col
    chunks interleaved with the x stream so the first matmul starts ~4us in.
  - q/k psums staged to SBUF by ScalarE copies (frees PSUM banks immediately;
    6-slot mm psum pool), RoPE on DVE reads the stage off the critical path.
  - exp on ScalarE writes p~ directly as bf16; PV matmul (free size 129) runs
    in bf16 at 1 cycle/row (fp32r would be 4x slower below 256 free).
  - o -> PE transpose in bf16 -> oT bf16 feeds the output projection (bf16
    weights) interleaved into head 3's attention, one t-range per i-block.
  - RoPE cos / sign-folded sin, causal 0/1 big-mask, identity are
    host-precomputed; consts stream in per-t-block chunks after the first
    weight chunks.
"""

import sys

import numpy as np
import ml_dtypes

for _p in ("/opt/trn_rl_repo", "/root/.axon_site/_ro/trn_rl_repo"):
    if _p not in sys.path:
        sys.path.append(_p)

import concourse.bass as bass
import concourse.bacc as bacc
import concourse.tile as tile
from concourse import mybir
from concourse.bass_utils import run_bass_kernel_spmd

F32 = mybir.dt.float32
BF16 = mybir.dt.bfloat16
AF = mybir.ActivationFunctionType

B, T, D, H = 2, 2048, 2048, 16
HPC = H // 4  # heads per core (4-way head TP)
DH = D // H   # 128
SCALE = float(DH) ** -0.5

TB = 512      # qkv-projection t-block (psum free width)
SB = 512      # attention i-block (score free width)


def build_nc(t=T, mm_dt="float32r", pv_dt="bfloat16"):
    """Build the SPMD per-core program.  `t` is the sequence length (smaller
    values are used for simulator validation)."""
    NT = t // 128    # token tiles
    TBE = min(TB, t)
    NTB = t // TBE   # qkv t-blocks
    sb = min(SB, t)
    NSB = t // sb    # attention i-blocks
    NIC = sb // 128  # i-chunks per i-block
    ND = D // 128    # contraction d-tiles
    C0 = sb - 128    # base column of the causal big-mask

    MDT = mybir.dt.float32r if (mm_dt == "float32r") else F32
    BDT = BF16 if (pv_dt == "bfloat16") else F32

    nc = bacc.Bacc("TRN2", target_bir_lowering=False, debug=False)

    xT = nc.dram_tensor("xT", [D, t], MDT, kind="ExternalInput").ap()
    wqh = nc.dram_tensor("wqh", [128, HPC * D], MDT, kind="ExternalInput").ap()
    wkh = nc.dram_tensor("wkh", [128, HPC * D], MDT, kind="ExternalInput").ap()
    wvh = nc.dram_tensor("wvh", [128, (HPC // 2) * 2 * D], MDT, kind="ExternalInput").ap()
    wpT = nc.dram_tensor("wpT", [HPC * DH, D], BDT, kind="ExternalInput").ap()
    cosT = nc.dram_tensor("cosT", [DH, t], BDT, kind="ExternalInput").ap()
    sinTm = nc.dram_tensor("sinTm", [DH, t], BDT, kind="ExternalInput").ap()
    bmask = nc.dram_tensor("bmask", [128, 128], BDT, kind="ExternalInput").ap()
    ident = nc.dram_tensor("ident", [128, 128], BDT, kind="ExternalInput").ap()
    rotm = nc.dram_tensor("rotm", [128, 128], BDT, kind="ExternalInput").ap()
    y = nc.dram_tensor("y", [t, D], F32, kind="ExternalOutput").ap()

    with tile.TileContext(nc) as tc:
        with (
            tc.tile_pool(name="consts", bufs=1) as cpool,
            tc.tile_pool(name="oTp", bufs=1) as opool,
            tc.tile_pool(name="qkp", bufs=2) as qkpool,
            tc.tile_pool(name="xtp", bufs=6) as xtp,
            tc.tile_pool(name="wqkp", bufs=1) as wqkp,
            tc.tile_pool(name="wvp", bufs=1) as wvp,
            tc.tile_pool(name="wpj", bufs=1) as wpj,
            tc.tile_pool(name="vep", bufs=2) as vep,
            tc.tile_pool(name="ptp", bufs=2 * NT + 2) as ptp,
            tc.tile_pool(name="tmpp", bufs=4) as tmpp,
            tc.tile_pool(name="smallp", bufs=6) as smallp,
            tc.tile_pool(name="ysp", bufs=6) as ysp,
            tc.tile_pool(name="ps_mm", bufs=6, space="PSUM") as ps_mm,
            tc.tile_pool(name="ps_sm", bufs=2, space="PSUM") as ps_sm,
        ):
            # const tiles allocated up front; DMAs issued later (chunked) so
            # the weight/x stream owns the head of the DMA queue.
            cos_sb = cpool.tile([DH, t], BDT, tag="cos")
            sin_sb = cpool.tile([DH, t], BDT, tag="sin")
            bm_sb = cpool.tile([128, 128], BDT, tag="bm")
            id_sb = cpool.tile([128, 128], BDT, tag="id")
            rot_sb = cpool.tile([128, 128], BDT, tag="rot")
            oT_sb = [
                opool.tile([DH, t], BDT, tag=f"oT{h}", name=f"oT{h}")
                for h in range(HPC)
            ]
            wp_sb = [
                wpj.tile([128, D], BDT, tag=f"wp{h}", name=f"wp{h}")
                for h in range(HPC)
            ]

            def rope_ip(qk, tb, name):
                """In-place RoPE on qk[:, tb-block] (holds the raw projection,
                parked there by a ScalarE psum copy).  The half-rotation (a
                cross-partition shuffle the DVE can't address) runs on the PE
                as a signed permutation matmul; the remaining DVE ops are all
                partition-aligned bf16 (2x mode)."""
                t0, t1_ = TBE * tb, TBE * (tb + 1)
                blk = qk[:, t0:t1_]
                rps = ps_mm.tile([128, TBE], F32, tag="mm", name=f"rot_{name}")
                nc.tensor.matmul(rps[:], rot_sb[:], blk, start=True, stop=True)
                r1 = tmpp.tile([128, TBE], BDT, tag="r1", name=f"r1_{name}")
                nc.vector.tensor_mul(r1[:], blk, cos_sb[:, t0:t1_])
                r2 = tmpp.tile([128, TBE], BDT, tag="r2", name=f"r2_{name}")
                nc.vector.tensor_mul(r2[:], rps[:], sin_sb[:, t0:t1_])
                nc.vector.tensor_add(blk, r1[:], r2[:])

            def emit_proj(tt, on_act=False):
                """y[128*tt:128*(tt+1), :] = sum_h oT_h[:, tt].T @ wp_h.
                on_act routes the psum drain to ScalarE (used for the blocks
                issued ahead of B chunks, whose DVE normalizes must not queue
                behind these copies)."""
                for db in range(D // 512):
                    ps = ps_mm.tile([128, 512], F32, tag="mm", name=f"psy{tt}_{db}")
                    for h in range(HPC):
                        nc.tensor.matmul(
                            ps[:],
                            oT_sb[h][:, 128 * tt:128 * (tt + 1)],
                            wp_sb[h][:, 512 * db:512 * (db + 1)],
                            start=(h == 0), stop=(h == HPC - 1))
                    yst = ysp.tile([128, 512], F32, tag="yst", name=f"yst{tt}_{db}")
                    if on_act:
                        nc.scalar.copy(yst[:], ps[:])
                    else:
                        nc.vector.tensor_copy(yst[:], ps[:])
                    nc.sync.dma_start(
                        y[128 * tt:128 * (tt + 1), 512 * db:512 * (db + 1)], yst[:])

            pair_bufs = {}
            pend_ropes = []

            def qkv_pair_steps(p2):
                """Generator: pair-p2 weight DMAs + qkv projection; yields
                once per t-block so attention chunks of the previous pair can
                interleave into the PE stream."""
                h = 2 * p2
                q_sbs = [None, None]
                k_sbs = [None, None]
                vext = [None, None]
                wq0 = wqkp.tile([128, D], MDT, tag="wq0", name=f"wq0_{p2}")
                wk0 = wqkp.tile([128, D], MDT, tag="wk0", name=f"wk0_{p2}")
                wq1 = wqkp.tile([128, D], MDT, tag="wq1", name=f"wq1_{p2}")
                wk1 = wqkp.tile([128, D], MDT, tag="wk1", name=f"wk1_{p2}")
                wv_sb = wvp.tile([128, 2 * D], MDT, tag="wv", name=f"wv{p2}")
                vext[0] = vep.tile([128, NT * 129], BDT, tag="ve0", name=f"ve0_{p2}")
                vext[1] = vep.tile([128, NT * 129], BDT, tag="ve1", name=f"ve1_{p2}")
                nc.vector.memset(vext[0][:], 1.0)
                nc.vector.memset(vext[1][:], 1.0)
                q_sbs[0] = qkpool.tile([DH, t], BDT, tag="q0", name=f"q0_{p2}")
                k_sbs[0] = qkpool.tile([DH, t], BDT, tag="k0", name=f"k0_{p2}")
                q_sbs[1] = qkpool.tile([DH, t], BDT, tag="q1", name=f"q1_{p2}")
                k_sbs[1] = qkpool.tile([DH, t], BDT, tag="k1", name=f"k1_{p2}")
                pair_bufs[p2] = {"q": q_sbs, "k": k_sbs, "ve": vext}

                if p2 == 1:
                    # prefetch the projection weights during pair-1 qkv
                    for hh in range(HPC):
                        nc.sync.dma_start(wp_sb[hh][:], wpT[128 * hh:128 * (hh + 1), :])

                # ---- qkv projection for the pair (one xT pass) ----
                ttpb = TBE // 128
                nvp = (ttpb + 1) // 2
                for tb in range(NTB):
                    t0, t1_ = TBE * tb, TBE * (tb + 1)
                    ps_q0 = ps_mm.tile([128, TBE], F32, tag="mm", name=f"psq0_{p2}_{tb}")
                    ps_k0 = ps_mm.tile([128, TBE], F32, tag="mm", name=f"psk0_{p2}_{tb}")
                    ps_q1 = ps_mm.tile([128, TBE], F32, tag="mm", name=f"psq1_{p2}_{tb}")
                    ps_k1 = ps_mm.tile([128, TBE], F32, tag="mm", name=f"psk1_{p2}_{tb}")
                    ps_vs = [
                        ps_sm.tile([128, 512], F32, tag="sm", name=f"psv{p2}_{tb}_{i}")
                        for i in range(nvp)
                    ]
                    for di in range(ND):
                        d0, d1 = 128 * di, 128 * (di + 1)
                        # weight chunks interleaved with the x stream so the
                        # first matmuls start after ~0.5MB of DMA
                        wq_chunks = []
                        if tb == 0 and di % 4 == 0:
                            wq_chunks = [(512 * (di // 4), 512 * (di // 4 + 1))]
                        for c0_, c1_ in wq_chunks:
                            nc.sync.dma_start(wq0[:, c0_:c1_], wqh[:, D * h + c0_:D * h + c1_])
                        xt_t = xtp.tile([128, TBE], MDT, tag="xt", name=f"xt{p2}_{tb}_{di}")
                        nc.sync.dma_start(xt_t[:], xT[d0:d1, t0:t1_])
                        for c0_, c1_ in wq_chunks:
                            nc.sync.dma_start(wk0[:, c0_:c1_], wkh[:, D * h + c0_:D * h + c1_])
                            nc.sync.dma_start(wq1[:, c0_:c1_], wqh[:, D * (h + 1) + c0_:D * (h + 1) + c1_])
                            nc.sync.dma_start(wk1[:, c0_:c1_], wkh[:, D * (h + 1) + c0_:D * (h + 1) + c1_])
                        if tb == 0 and di % 2 == 0:
                            v0, v1 = 512 * (di // 2), 512 * (di // 2 + 1)
                            nc.sync.dma_start(wv_sb[:, v0:v1], wvh[:, 2 * D * p2 + v0:2 * D * p2 + v1])
                        st, sp = di == 0, di == ND - 1
                        nc.tensor.matmul(ps_q0[:], wq0[:, d0:d1], xt_t[:], start=st, stop=sp)
                        nc.tensor.matmul(ps_k0[:], wk0[:, d0:d1], xt_t[:], start=st, stop=sp)
                        nc.tensor.matmul(ps_q1[:], wq1[:, d0:d1], xt_t[:], start=st, stop=sp)
                        nc.tensor.matmul(ps_k1[:], wk1[:, d0:d1], xt_t[:], start=st, stop=sp)
                        for tt in range(ttpb):
                            # start=True clears the whole bank: only the
                            # even half may set it; the odd half's first
                            # write lands via has_written=0 overwrite.
                            nc.tensor.matmul(
                                ps_vs[tt // 2][:, 256 * (tt % 2):256 * (tt % 2) + 256],
                                xt_t[:, 128 * tt:128 * (tt + 1)],
                                wv_sb[:, 256 * di:256 * (di + 1)],
                                start=(st and tt % 2 == 0), stop=sp,
                                skip_group_check=True)
                    if p2 == 0:
                        # stream the consts the tb's ropes need right
                        # behind its x tiles
                        nc.sync.dma_start(cos_sb[:, t0:t1_], cosT[:, t0:t1_])
                        nc.sync.dma_start(sin_sb[:, t0:t1_], sinTm[:, t0:t1_])
                        if tb == 0:
                            nc.sync.dma_start(rot_sb[:], rotm[:])
                        if tb == 1:
                            nc.sync.dma_start(bm_sb[:], bmask[:])
                            nc.sync.dma_start(id_sb[:], ident[:])
                    # park raw psums straight into the q/k tiles on
                    # ScalarE (frees the banks without waiting on DVE);
                    # RoPE rotates in place later.  At pair 0's last t-block
                    # the first attention chunks are waiting on these slots,
                    # so split the copies across ScalarE and DVE.
                    split = tb == NTB - 1
                    for i_, (ps, dst) in enumerate(((ps_q0, q_sbs[0]), (ps_k0, k_sbs[0]), (ps_q1, q_sbs[1]), (ps_k1, k_sbs[1]))):
                        if split and i_ % 2 == 1:
                            nc.vector.tensor_copy(dst[:, t0:t1_], ps[:])
                        else:
                            nc.scalar.copy(dst[:, t0:t1_], ps[:])
                    for tt in range(ttpb):
                        gt = tb * ttpb + tt
                        o0 = 256 * (tt % 2)
                        nc.scalar.copy(
                            vext[0][:, 129 * gt:129 * gt + 128],
                            ps_vs[tt // 2][:, o0:o0 + 128])
                        nc.scalar.copy(
                            vext[1][:, 129 * gt:129 * gt + 128],
                            ps_vs[tt // 2][:, o0 + 128:o0 + 256])
                    # RoPE policy: pair 0 defers q1/k1 + the last tb's q0/k0
                    # into the attention stream (so the pair-boundary DVE
                    # queue starts with the diagonal masks).  Pair 1's tb
                    # loop is already interleaved with h0/h1 attention, so
                    # in-loop ropes hide in the t-block gaps; only the last
                    # tb's four ropes defer (consumed by h2's chunks).
                    last = tb == NTB - 1
                    if not last:
                        rope_ip(q_sbs[0], tb, f"q0_{p2}_{tb}")
                        rope_ip(k_sbs[0], tb, f"k0_{p2}_{tb}")
                        if p2 == 1:
                            rope_ip(q_sbs[1], tb, f"q1_{p2}_{tb}")
                            rope_ip(k_sbs[1], tb, f"k1_{p2}_{tb}")
                    else:
                        pend_ropes.append((q_sbs[0], tb, f"q0_{p2}_{tb}"))
                        pend_ropes.append((k_sbs[0], tb, f"k0_{p2}_{tb}"))
                        pend_ropes.append((q_sbs[1], tb, f"q1_{p2}_{tb}"))
                        pend_ropes.append((k_sbs[1], tb, f"k1_{p2}_{tb}"))
                    if p2 == 0 and not last:
                        pend_ropes.append((q_sbs[1], tb, f"q1_{p2}_{tb}"))
                        pend_ropes.append((k_sbs[1], tb, f"k1_{p2}_{tb}"))
                    yield

            def attention_steps(h, quotas, pipelined, do_proj=False):
                """Generator for head h's attention, yielding once per chunk.
                pipelined=True: A0 A1 B0 A2 B1 A3 B2 B3 (B lags A by one).
                pipelined=False: A0 B0 A1 B1 ... (B right after its A).
                quotas[i] deferred ropes are issued after the i-th B chunk."""
                par = h % 2
                bufs = pair_bufs[h // 2]
                q_sb, k_sb = bufs["q"][par], bufs["k"][par]
                ve = bufs["ve"][par]
                if par == 0 and pend_ropes:
                    # rotate this head's q/k first (its own ib3 needs them)
                    mine = [e for e in pend_ropes if e[0] is q_sb or e[0] is k_sb]
                    rest = [e for e in pend_ropes if not (e[0] is q_sb or e[0] is k_sb)]
                    pend_ropes[:] = mine + rest

                def stage_a(ib):
                    """scores + exp + diagonal mask for i-block ib."""
                    i0 = sb * ib
                    jt_max = (i0 + sb) // 128 - 1  # inclusive
                    pts = [None] * (jt_max + 1)
                    for jt in range(jt_max + 1):
                        s_ps = ps_mm.tile([128, sb], F32, tag="mm", name=f"s{h}_{ib}_{jt}")
                        nc.tensor.matmul(
                            s_ps[:],
                            k_sb[:, 128 * jt:128 * (jt + 1)],
                            q_sb[:, i0:i0 + sb],
                            start=True, stop=True)
                        pt_t = ptp.tile([128, sb], BDT, tag="pt", name=f"pt{h}_{ib}_{jt}")
                        nc.scalar.activation(pt_t[:], s_ps[:], AF.Exp, scale=SCALE)
                        m = jt - NIC * ib
                        if m >= 0:
                            # only the 128x128 sub-block straddling the causal
                            # diagonal needs masking: fully-masked sub-blocks
                            # are never read by the PV loop bounds in stage_b
                            pm = pt_t[:, 128 * m:128 * (m + 1)]
                            nc.vector.tensor_mul(pm, pm, bm_sb[:])
                        pts[jt] = pt_t
                    return pts

                def stage_b(ib, pts, bi):
                    """PV + normalize + transpose for i-block ib."""
                    i0 = sb * ib

                    def finish(ic, pv):
                        rc = smallp.tile([128, 1], F32, tag="rc", name=f"rc{h}_{ib}_{ic}")
                        nc.vector.reciprocal(rc[:], pv[:, 128:129])
                        o_sb = smallp.tile([128, 128], BDT, tag="o", name=f"o{h}_{ib}_{ic}")
                        nc.vector.tensor_scalar_mul(o_sb[:], pv[:, 0:128], rc[:])
                        ot_ps = ps_mm.tile([128, 128], BDT, tag="mm", name=f"otp{h}_{ib}_{ic}")
                        nc.tensor.transpose(ot_ps[:], o_sb[:], id_sb[:])
                        c0 = i0 + 128 * ic
                        nc.vector.tensor_copy(oT_sb[h][:, c0:c0 + 128], ot_ps[:])

                    prev = None
                    for ic in range(NIC):
                        last_jt = NIC * ib + ic
                        pv = ps_sm.tile([128, 129], F32, tag="sm", name=f"pv{h}_{ib}_{ic}")
                        for jt in range(last_jt + 1):
                            nc.tensor.matmul(
                                pv[:],
                                pts[jt][:, 128 * ic:128 * (ic + 1)],
                                ve[:, 129 * jt:129 * (jt + 1)],
                                start=(jt == 0), stop=(jt == last_jt))
                        if prev is not None:
                            finish(*prev)
                        prev = (ic, pv)
                    finish(*prev)
                    if do_proj:
                        # head 3's oT columns complete per i-block: fold the
                        # output projection for those t-tiles in right away
                        # (the last i-block drains on both engines so the
                        # kernel tail is DMA-limited, not copy-limited)
                        for tt in range(NIC * ib, NIC * (ib + 1)):
                            emit_proj(tt, on_act=(ib == NSB - 1 and tt % 2 == 0))
                    # deferred RoPEs enter the DVE queue after the B chunk so
                    # the masks and PV-psum-freeing normalizes stay ahead
                    consume_ropes(quotas[bi])

                if pipelined:
                    pts_prev = None
                    for ib in range(NSB):
                        pts_cur = stage_a(ib)
                        yield
                        if pts_prev is not None:
                            stage_b(ib - 1, pts_prev, ib - 1)
                            yield
                        pts_prev = pts_cur
                    stage_b(NSB - 1, pts_prev, NSB - 1)
                    yield
                else:
                    for ib in range(NSB):
                        pts = stage_a(ib)
                        yield
                        stage_b(ib, pts, ib)
                        yield

            def stepn(g, n):
                for _ in range(n):
                    next(g)

            def consume_ropes(n):
                for qk, tb_, nm_ in pend_ropes[:n]:
                    rope_ip(qk, tb_, nm_)
                del pend_ropes[:n]

            # ---- schedule -------------------------------------------
            # pair 0 qkv alone (nothing to overlap), then h0/h1 attention
            # chunks interleaved into pair 1's t-blocks (attention is
            # exp-bound on ScalarE; the qkv matmuls keep PE busy), then
            # h2/h3 attention interleaved with the output projection.
            for _ in qkv_pair_steps(0):
                pass
            for _ in attention_steps(0, [3, 3, 2, 2], True):
                pass
            for _ in attention_steps(1, [0, 0, 0, 0], True):
                pass
            for _ in qkv_pair_steps(1):
                pass
            for _ in attention_steps(2, [2, 2, 0, 0], True):
                pass
            for _ in attention_steps(3, [0, 0, 0, 0], True, do_proj=True):
                pass

    nc.compile()
    return nc


def host_consts(t=T):
    """RoPE cos/sin (f32, matching the jax reference), causal big-mask, identity."""
    inv = (1.0 / (np.float32(10000.0) ** (np.arange(0, DH, 2, dtype=np.float32) / np.float32(DH)))).astype(np.float32)
    tt = np.arange(t, dtype=np.float32)
    fr = np.outer(tt, inv).astype(np.float32)       # [t, 64]
    emb = np.concatenate([fr, fr], axis=1)          # [t, 128]
    cosT = np.ascontiguousarray(np.cos(emb).T.astype(np.float32))
    sinTm = np.ascontiguousarray(np.sin(emb).T.astype(np.float32))
    jj = np.arange(128)[:, None]
    cc = np.arange(128)[None, :]
    bmask = (cc >= jj).astype(np.float32)
    ident = np.eye(128, dtype=np.float32)
    # signed half-rotation: (rotm.T @ x)[d] = -x[d+64] for d<64, x[d-64] else
    rotm = np.zeros((128, 128), dtype=np.float32)
    for d in range(64):
        rotm[d + 64, d] = -1.0
        rotm[d, d + 64] = 1.0
    return cosT, sinTm, bmask, ident, rotm


def _warrange(w):
    """[128*nh rows, D] head-major weight slice -> [128, nh*D] sbuf-ready layout:
    block h, col di*128+c of partition p  =  w[128*h + c, 128*di + p]."""
    nh = w.shape[0] // 128
    d = w.shape[1]
    out = np.empty((128, nh * d), dtype=np.float32)
    for h in range(nh):
        a = w[128 * h:128 * (h + 1), :].T.reshape(d // 128, 128, 128)  # [di, p, c]
        out[:, d * h:d * (h + 1)] = a.transpose(1, 0, 2).reshape(128, d)
    return out


def _wvarrange(w):
    """[512 rows, D] 4-head v-weights -> [128, 2*2*D]: per pair, di-major blocks of
    [even-head 128 cols | odd-head 128 cols]."""
    d = w.shape[1]
    blocks = []
    for p2 in range(2):
        e = w[256 * p2:256 * p2 + 128, :].T.reshape(d // 128, 128, 128)
        o = w[256 * p2 + 128:256 * p2 + 256, :].T.reshape(d // 128, 128, 128)
        pair = np.concatenate([e, o], axis=2)          # [di, p, 256]
        blocks.append(pair.transpose(1, 0, 2).reshape(128, 2 * d))
    return np.concatenate(blocks, axis=1)


def shard_inputs(x, w_qkv, w_proj, t=T, pv_dt="bfloat16"):
    """Build the 8 per-core input maps."""
    bdt = ml_dtypes.bfloat16 if pv_dt == "bfloat16" else np.float32
    cosT, sinTm, bmask, ident, rotm = host_consts(t)
    cosT = cosT.astype(bdt)
    sinTm = sinTm.astype(bdt)
    bmask = bmask.astype(bdt)
    ident = ident.astype(bdt)
    rotm = rotm.astype(bdt)
    d = x.shape[2]
    in_maps = []
    for c in range(8):
        b, g = divmod(c, 4)
        s0, s1 = 512 * g, 512 * (g + 1)
        in_maps.append(dict(
            xT=np.ascontiguousarray(x[b].T),
            wqh=_warrange(w_qkv[s0:s1, :]),
            wkh=_warrange(w_qkv[d + s0:d + s1, :]),
            wvh=_wvarrange(w_qkv[2 * d + s0:2 * d + s1, :]),
            wpT=np.ascontiguousarray(w_proj[:, s0:s1].T).astype(bdt),
            cosT=cosT, sinTm=sinTm, bmask=bmask, ident=ident, rotm=rotm,
        ))
    return in_maps


_NC_CACHE = {}


def get_nc(t=T, mm_dt="float32r", pv_dt="bfloat16"):
    key = (t, mm_dt, pv_dt)
    if key not in _NC_CACHE:
        _NC_CACHE[key] = build_nc(t=t, mm_dt=mm_dt, pv_dt=pv_dt)
    return _NC_CACHE[key]


def kernel(x, w_qkv, w_proj):
    x = np.asarray(x, dtype=np.float32)
    w_qkv = np.asarray(w_qkv, dtype=np.float32)
    w_proj = np.asarray(w_proj, dtype=np.float32)
    b_, t_, d_ = x.shape
    in_maps = shard_inputs(x, w_qkv, w_proj, t=t_)
    nc = get_nc(t=t_)
    res = run_bass_kernel_spmd(nc, in_maps, list(range(8))).results
    out = np.zeros((b_, t_, d_), dtype=np.float32)
    for c in range(8):
        b, _ = divmod(c, 4)
        out[b] += res[c]["y"]
    return out


# revision 58
# speedup vs baseline: 1.0229x; 1.0137x over previous
"""Trainium2 Bass kernel: causal self-attention with RoPE (B=2, T=2048, D=2048, H=16).

Sharding: 8 cores = 2-way data parallel over batch x 4-way tensor parallel over
heads.  Core c = 4*b + g computes batch b, heads 4g..4g+3, and produces a
partial output y_partial = attn_out[:, heads_g] @ w_proj[:, heads_g].T which the
host sums over g.

Per-core pipeline (v2 — tuned against the TimelineSim cost model):
  - qkv projection in fp32r (full-rate at free>=256), weights DMA'd in 512-col
    chunks interleaved with the x stream so the first matmul starts ~4us in.
  - q/k psums staged to SBUF by ScalarE copies (frees PSUM banks immediately;
    6-slot mm psum pool), RoPE on DVE reads the stage off the critical path.
  - exp on ScalarE writes p~ directly as bf16; PV matmul (free size 129) runs
    in bf16 at 1 cycle/row (fp32r would be 4x slower below 256 free).
  - o -> PE transpose in bf16 -> oT bf16 feeds the output projection (bf16
    weights) interleaved into head 3's attention, one t-range per i-block.
  - RoPE cos / sign-folded sin, causal 0/1 big-mask, identity are
    host-precomputed; consts stream in per-t-block chunks after the first
    weight chunks.
"""

import sys

import numpy as np
import ml_dtypes

for _p in ("/opt/trn_rl_repo", "/root/.axon_site/_ro/trn_rl_repo"):
    if _p not in sys.path:
        sys.path.append(_p)

import concourse.bass as bass
import concourse.bacc as bacc
import concourse.tile as tile
from concourse import mybir
from concourse.bass_utils import run_bass_kernel_spmd

F32 = mybir.dt.float32
BF16 = mybir.dt.bfloat16
AF = mybir.ActivationFunctionType

B, T, D, H = 2, 2048, 2048, 16
HPC = H // 4  # heads per core (4-way head TP)
DH = D // H   # 128
SCALE = float(DH) ** -0.5

TB = 512      # qkv-projection t-block (psum free width)
SB = 512      # attention i-block (score free width)


def build_nc(t=T, mm_dt="float32r", pv_dt="bfloat16"):
    """Build the SPMD per-core program.  `t` is the sequence length (smaller
    values are used for simulator validation)."""
    NT = t // 128    # token tiles
    TBE = min(TB, t)
    NTB = t // TBE   # qkv t-blocks
    sb = min(SB, t)
    NSB = t // sb    # attention i-blocks
    NIC = sb // 128  # i-chunks per i-block
    ND = D // 128    # contraction d-tiles
    C0 = sb - 128    # base column of the causal big-mask

    MDT = mybir.dt.float32r if (mm_dt == "float32r") else F32
    BDT = BF16 if (pv_dt == "bfloat16") else F32

    nc = bacc.Bacc("TRN2", target_bir_lowering=False, debug=False)

    xT = nc.dram_tensor("xT", [D, t], MDT, kind="ExternalInput").ap()
    wqh = nc.dram_tensor("wqh", [128, HPC * D], MDT, kind="ExternalInput").ap()
    wkh = nc.dram_tensor("wkh", [128, HPC * D], MDT, kind="ExternalInput").ap()
    wvh = nc.dram_tensor("wvh", [128, (HPC // 2) * 2 * D], MDT, kind="ExternalInput").ap()
    wpT = nc.dram_tensor("wpT", [HPC * DH, D], BDT, kind="ExternalInput").ap()
    cosT = nc.dram_tensor("cosT", [DH, t], BDT, kind="ExternalInput").ap()
    sinTm = nc.dram_tensor("sinTm", [DH, t], BDT, kind="ExternalInput").ap()
    bmask = nc.dram_tensor("bmask", [128, 128], BDT, kind="ExternalInput").ap()
    ident = nc.dram_tensor("ident", [128, 128], BDT, kind="ExternalInput").ap()
    rotm = nc.dram_tensor("rotm", [128, 128], BDT, kind="ExternalInput").ap()
    y = nc.dram_tensor("y", [t, D], F32, kind="ExternalOutput").ap()

    with tile.TileContext(nc) as tc:
        with (
            tc.tile_pool(name="consts", bufs=1) as cpool,
            tc.tile_pool(name="oTp", bufs=1) as opool,
            tc.tile_pool(name="qkp", bufs=2) as qkpool,
            tc.tile_pool(name="xtp", bufs=6) as xtp,
            tc.tile_pool(name="wqkp", bufs=1) as wqkp,
            tc.tile_pool(name="wvp", bufs=1) as wvp,
            tc.tile_pool(name="wpj", bufs=1) as wpj,
            tc.tile_pool(name="vep", bufs=2) as vep,
            tc.tile_pool(name="ptp", bufs=2 * NT + 2) as ptp,
            tc.tile_pool(name="tmpp", bufs=4) as tmpp,
            tc.tile_pool(name="smallp", bufs=6) as smallp,
            tc.tile_pool(name="ysp", bufs=6) as ysp,
            tc.tile_pool(name="ps_mm", bufs=6, space="PSUM") as ps_mm,
            tc.tile_pool(name="ps_sm", bufs=2, space="PSUM") as ps_sm,
        ):
            # const tiles allocated up front; DMAs issued later (chunked) so
            # the weight/x stream owns the head of the DMA queue.
            cos_sb = cpool.tile([DH, t], BDT, tag="cos")
            sin_sb = cpool.tile([DH, t], BDT, tag="sin")
            bm_sb = cpool.tile([128, 128], BDT, tag="bm")
            id_sb = cpool.tile([128, 128], BDT, tag="id")
            rot_sb = cpool.tile([128, 128], BDT, tag="rot")
            oT_sb = [
                opool.tile([DH, t], BDT, tag=f"oT{h}", name=f"oT{h}")
                for h in range(HPC)
            ]
            wp_sb = [
                wpj.tile([128, D], BDT, tag=f"wp{h}", name=f"wp{h}")
                for h in range(HPC)
            ]

            def rope_ip(qk, tb, name):
                """In-place RoPE on qk[:, tb-block] (holds the raw projection,
                parked there by a ScalarE psum copy).  The half-rotation (a
                cross-partition shuffle the DVE can't address) runs on the PE
                as a signed permutation matmul; the remaining DVE ops are all
                partition-aligned bf16 (2x mode)."""
                t0, t1_ = TBE * tb, TBE * (tb + 1)
                blk = qk[:, t0:t1_]
                rps = ps_mm.tile([128, TBE], F32, tag="mm", name=f"rot_{name}")
                nc.tensor.matmul(rps[:], rot_sb[:], blk, start=True, stop=True)
                r1 = tmpp.tile([128, TBE], BDT, tag="r1", name=f"r1_{name}")
                nc.vector.tensor_mul(r1[:], blk, cos_sb[:, t0:t1_])
                r2 = tmpp.tile([128, TBE], BDT, tag="r2", name=f"r2_{name}")
                nc.vector.tensor_mul(r2[:], rps[:], sin_sb[:, t0:t1_])
                nc.vector.tensor_add(blk, r1[:], r2[:])

            def emit_proj(tt, on_act=False):
                """y[128*tt:128*(tt+1), :] = sum_h oT_h[:, tt].T @ wp_h.
                on_act routes the psum drain to ScalarE (used for the blocks
                issued ahead of B chunks, whose DVE normalizes must not queue
                behind these copies)."""
                for db in range(D // 512):
                    ps = ps_mm.tile([128, 512], F32, tag="mm", name=f"psy{tt}_{db}")
                    for h in range(HPC):
                        nc.tensor.matmul(
                            ps[:],
                            oT_sb[h][:, 128 * tt:128 * (tt + 1)],
                            wp_sb[h][:, 512 * db:512 * (db + 1)],
                            start=(h == 0), stop=(h == HPC - 1))
                    yst = ysp.tile([128, 512], F32, tag="yst", name=f"yst{tt}_{db}")
                    if on_act:
                        nc.scalar.copy(yst[:], ps[:])
                    else:
                        nc.vector.tensor_copy(yst[:], ps[:])
                    nc.sync.dma_start(
                        y[128 * tt:128 * (tt + 1), 512 * db:512 * (db + 1)], yst[:])

            pair_bufs = {}
            pend_ropes = []

            def qkv_pair_steps(p2):
                """Generator: pair-p2 weight DMAs + qkv projection; yields
                once per t-block so attention chunks of the previous pair can
                interleave into the PE stream."""
                h = 2 * p2
                q_sbs = [None, None]
                k_sbs = [None, None]
                vext = [None, None]
                wq0 = wqkp.tile([128, D], MDT, tag="wq0", name=f"wq0_{p2}")
                wk0 = wqkp.tile([128, D], MDT, tag="wk0", name=f"wk0_{p2}")
                wq1 = wqkp.tile([128, D], MDT, tag="wq1", name=f"wq1_{p2}")
                wk1 = wqkp.tile([128, D], MDT, tag="wk1", name=f"wk1_{p2}")
                wv_sb = wvp.tile([128, 2 * D], MDT, tag="wv", name=f"wv{p2}")
                vext[0] = vep.tile([128, NT * 129], BDT, tag="ve0", name=f"ve0_{p2}")
                vext[1] = vep.tile([128, NT * 129], BDT, tag="ve1", name=f"ve1_{p2}")
                nc.vector.memset(vext[0][:], 1.0)
                nc.vector.memset(vext[1][:], 1.0)
                q_sbs[0] = qkpool.tile([DH, t], BDT, tag="q0", name=f"q0_{p2}")
                k_sbs[0] = qkpool.tile([DH, t], BDT, tag="k0", name=f"k0_{p2}")
                q_sbs[1] = qkpool.tile([DH, t], BDT, tag="q1", name=f"q1_{p2}")
                k_sbs[1] = qkpool.tile([DH, t], BDT, tag="k1", name=f"k1_{p2}")
                pair_bufs[p2] = {"q": q_sbs, "k": k_sbs, "ve": vext}

                if p2 == 1:
                    # prefetch the projection weights during pair-1 qkv
                    for hh in range(HPC):
                        nc.sync.dma_start(wp_sb[hh][:], wpT[128 * hh:128 * (hh + 1), :])

                # ---- qkv projection for the pair (one xT pass) ----
                ttpb = TBE // 128
                nvp = (ttpb + 1) // 2
                for tb in range(NTB):
                    t0, t1_ = TBE * tb, TBE * (tb + 1)
                    ps_q0 = ps_mm.tile([128, TBE], F32, tag="mm", name=f"psq0_{p2}_{tb}")
                    ps_k0 = ps_mm.tile([128, TBE], F32, tag="mm", name=f"psk0_{p2}_{tb}")
                    ps_q1 = ps_mm.tile([128, TBE], F32, tag="mm", name=f"psq1_{p2}_{tb}")
                    ps_k1 = ps_mm.tile([128, TBE], F32, tag="mm", name=f"psk1_{p2}_{tb}")
                    ps_vs = [
                        ps_sm.tile([128, 512], F32, tag="sm", name=f"psv{p2}_{tb}_{i}")
                        for i in range(nvp)
                    ]
                    for di in range(ND):
                        d0, d1 = 128 * di, 128 * (di + 1)
                        # weight chunks interleaved with the x stream so the
                        # first matmuls start after ~0.5MB of DMA
                        wq_chunks = []
                        if tb == 0 and di % 4 == 0:
                            wq_chunks = [(512 * (di // 4), 512 * (di // 4 + 1))]
                        for c0_, c1_ in wq_chunks:
                            nc.sync.dma_start(wq0[:, c0_:c1_], wqh[:, D * h + c0_:D * h + c1_])
                        xt_t = xtp.tile([128, TBE], MDT, tag="xt", name=f"xt{p2}_{tb}_{di}")
                        nc.sync.dma_start(xt_t[:], xT[d0:d1, t0:t1_])
                        for c0_, c1_ in wq_chunks:
                            nc.sync.dma_start(wk0[:, c0_:c1_], wkh[:, D * h + c0_:D * h + c1_])
                            nc.sync.dma_start(wq1[:, c0_:c1_], wqh[:, D * (h + 1) + c0_:D * (h + 1) + c1_])
                            nc.sync.dma_start(wk1[:, c0_:c1_], wkh[:, D * (h + 1) + c0_:D * (h + 1) + c1_])
                        if tb == 0 and di % 2 == 0:
                            v0, v1 = 512 * (di // 2), 512 * (di // 2 + 1)
                            nc.sync.dma_start(wv_sb[:, v0:v1], wvh[:, 2 * D * p2 + v0:2 * D * p2 + v1])
                        st, sp = di == 0, di == ND - 1
                        nc.tensor.matmul(ps_q0[:], wq0[:, d0:d1], xt_t[:], start=st, stop=sp)
                        nc.tensor.matmul(ps_k0[:], wk0[:, d0:d1], xt_t[:], start=st, stop=sp)
                        nc.tensor.matmul(ps_q1[:], wq1[:, d0:d1], xt_t[:], start=st, stop=sp)
                        nc.tensor.matmul(ps_k1[:], wk1[:, d0:d1], xt_t[:], start=st, stop=sp)
                        for tt in range(ttpb):
                            # start=True clears the whole bank: only the
                            # even half may set it; the odd half's first
                            # write lands via has_written=0 overwrite.
                            nc.tensor.matmul(
                                ps_vs[tt // 2][:, 256 * (tt % 2):256 * (tt % 2) + 256],
                                xt_t[:, 128 * tt:128 * (tt + 1)],
                                wv_sb[:, 256 * di:256 * (di + 1)],
                                start=(st and tt % 2 == 0), stop=sp,
                                skip_group_check=True)
                    if p2 == 0:
                        # stream the consts the tb's ropes need right
                        # behind its x tiles
                        nc.sync.dma_start(cos_sb[:, t0:t1_], cosT[:, t0:t1_])
                        nc.sync.dma_start(sin_sb[:, t0:t1_], sinTm[:, t0:t1_])
                        if tb == 0:
                            nc.sync.dma_start(rot_sb[:], rotm[:])
                        if tb == 1:
                            nc.sync.dma_start(bm_sb[:], bmask[:])
                            nc.sync.dma_start(id_sb[:], ident[:])
                    # park raw psums straight into the q/k tiles on
                    # ScalarE (frees the banks without waiting on DVE);
                    # RoPE rotates in place later.  At pair 0's last t-block
                    # the first attention chunks are waiting on these slots,
                    # so split the copies across ScalarE and DVE.
                    split = tb == NTB - 1
                    for i_, (ps, dst) in enumerate(((ps_q0, q_sbs[0]), (ps_k0, k_sbs[0]), (ps_q1, q_sbs[1]), (ps_k1, k_sbs[1]))):
                        if split and i_ % 2 == 1:
                            nc.vector.tensor_copy(dst[:, t0:t1_], ps[:])
                        else:
                            nc.scalar.copy(dst[:, t0:t1_], ps[:])
                    vcopy = nc.vector.tensor_copy if split else nc.scalar.copy
                    for tt in range(ttpb):
                        gt = tb * ttpb + tt
                        o0 = 256 * (tt % 2)
                        vcopy(
                            vext[0][:, 129 * gt:129 * gt + 128],
                            ps_vs[tt // 2][:, o0:o0 + 128])
                        vcopy(
                            vext[1][:, 129 * gt:129 * gt + 128],
                            ps_vs[tt // 2][:, o0 + 128:o0 + 256])
                    # RoPE policy: pair 0 defers q1/k1 + the last tb's q0/k0
                    # into the attention stream (so the pair-boundary DVE
                    # queue starts with the diagonal masks).  Pair 1's tb
                    # loop is already interleaved with h0/h1 attention, so
                    # in-loop ropes hide in the t-block gaps; only the last
                    # tb's four ropes defer (consumed by h2's chunks).
                    last = tb == NTB - 1
                    if not last:
                        rope_ip(q_sbs[0], tb, f"q0_{p2}_{tb}")
                        rope_ip(k_sbs[0], tb, f"k0_{p2}_{tb}")
                        if p2 == 1:
                            rope_ip(q_sbs[1], tb, f"q1_{p2}_{tb}")
                            rope_ip(k_sbs[1], tb, f"k1_{p2}_{tb}")
                    else:
                        pend_ropes.append((q_sbs[0], tb, f"q0_{p2}_{tb}"))
                        pend_ropes.append((k_sbs[0], tb, f"k0_{p2}_{tb}"))
                        pend_ropes.append((q_sbs[1], tb, f"q1_{p2}_{tb}"))
                        pend_ropes.append((k_sbs[1], tb, f"k1_{p2}_{tb}"))
                    if p2 == 0 and not last:
                        pend_ropes.append((q_sbs[1], tb, f"q1_{p2}_{tb}"))
                        pend_ropes.append((k_sbs[1], tb, f"k1_{p2}_{tb}"))
                    yield

            def attention_steps(h, quotas, pipelined, do_proj=False):
                """Generator for head h's attention, yielding once per chunk.
                pipelined=True: A0 A1 B0 A2 B1 A3 B2 B3 (B lags A by one).
                pipelined=False: A0 B0 A1 B1 ... (B right after its A).
                quotas[i] deferred ropes are issued after the i-th B chunk."""
                par = h % 2
                bufs = pair_bufs[h // 2]
                q_sb, k_sb = bufs["q"][par], bufs["k"][par]
                ve = bufs["ve"][par]
                if par == 0 and pend_ropes:
                    # rotate this head's q/k first (its own ib3 needs them)
                    mine = [e for e in pend_ropes if e[0] is q_sb or e[0] is k_sb]
                    rest = [e for e in pend_ropes if not (e[0] is q_sb or e[0] is k_sb)]
                    pend_ropes[:] = mine + rest

                def stage_a(ib):
                    """scores + exp + diagonal mask for i-block ib."""
                    i0 = sb * ib
                    jt_max = (i0 + sb) // 128 - 1  # inclusive
                    pts = [None] * (jt_max + 1)
                    for jt in range(jt_max + 1):
                        s_ps = ps_mm.tile([128, sb], F32, tag="mm", name=f"s{h}_{ib}_{jt}")
                        nc.tensor.matmul(
                            s_ps[:],
                            k_sb[:, 128 * jt:128 * (jt + 1)],
                            q_sb[:, i0:i0 + sb],
                            start=True, stop=True)
                        pt_t = ptp.tile([128, sb], BDT, tag="pt", name=f"pt{h}_{ib}_{jt}")
                        nc.scalar.activation(pt_t[:], s_ps[:], AF.Exp, scale=SCALE)
                        m = jt - NIC * ib
                        if m >= 0:
                            # only the 128x128 sub-block straddling the causal
                            # diagonal needs masking: fully-masked sub-blocks
                            # are never read by the PV loop bounds in stage_b
                            pm = pt_t[:, 128 * m:128 * (m + 1)]
                            nc.vector.tensor_mul(pm, pm, bm_sb[:])
                        pts[jt] = pt_t
                    return pts

                def stage_b(ib, pts, bi):
                    """PV + normalize + transpose for i-block ib."""
                    i0 = sb * ib

                    def finish(ic, pv):
                        rc = smallp.tile([128, 1], F32, tag="rc", name=f"rc{h}_{ib}_{ic}")
                        nc.vector.reciprocal(rc[:], pv[:, 128:129])
                        o_sb = smallp.tile([128, 128], BDT, tag="o", name=f"o{h}_{ib}_{ic}")
                        nc.vector.tensor_scalar_mul(o_sb[:], pv[:, 0:128], rc[:])
                        ot_ps = ps_mm.tile([128, 128], BDT, tag="mm", name=f"otp{h}_{ib}_{ic}")
                        nc.tensor.transpose(ot_ps[:], o_sb[:], id_sb[:])
                        c0 = i0 + 128 * ic
                        nc.vector.tensor_copy(oT_sb[h][:, c0:c0 + 128], ot_ps[:])

                    prev = None
                    for ic in range(NIC):
                        last_jt = NIC * ib + ic
                        pv = ps_sm.tile([128, 129], F32, tag="sm", name=f"pv{h}_{ib}_{ic}")
                        for jt in range(last_jt + 1):
                            nc.tensor.matmul(
                                pv[:],
                                pts[jt][:, 128 * ic:128 * (ic + 1)],
                                ve[:, 129 * jt:129 * (jt + 1)],
                                start=(jt == 0), stop=(jt == last_jt))
                        if prev is not None:
                            finish(*prev)
                        prev = (ic, pv)
                    finish(*prev)
                    if do_proj:
                        # head 3's oT columns complete per i-block: fold the
                        # output projection for those t-tiles in right away
                        # (the last i-block drains on both engines so the
                        # kernel tail is DMA-limited, not copy-limited)
                        for tt in range(NIC * ib, NIC * (ib + 1)):
                            emit_proj(tt, on_act=(ib == NSB - 1 and tt % 2 == 0))
                    # deferred RoPEs enter the DVE queue after the B chunk so
                    # the masks and PV-psum-freeing normalizes stay ahead
                    consume_ropes(quotas[bi])

                if pipelined:
                    pts_prev = None
                    for ib in range(NSB):
                        pts_cur = stage_a(ib)
                        yield
                        if pts_prev is not None:
                            stage_b(ib - 1, pts_prev, ib - 1)
                            yield
                        pts_prev = pts_cur
                    stage_b(NSB - 1, pts_prev, NSB - 1)
                    yield
                else:
                    for ib in range(NSB):
                        pts = stage_a(ib)
                        yield
                        stage_b(ib, pts, ib)
                        yield

            def stepn(g, n):
                for _ in range(n):
                    next(g)

            def consume_ropes(n):
                for qk, tb_, nm_ in pend_ropes[:n]:
                    rope_ip(qk, tb_, nm_)
                del pend_ropes[:n]

            # ---- schedule -------------------------------------------
            # pair 0 qkv alone (nothing to overlap), then h0/h1 attention
            # chunks interleaved into pair 1's t-blocks (attention is
            # exp-bound on ScalarE; the qkv matmuls keep PE busy), then
            # h2/h3 attention interleaved with the output projection.
            for _ in qkv_pair_steps(0):
                pass
            for _ in attention_steps(0, [3, 3, 2, 2], True):
                pass
            for _ in attention_steps(1, [0, 0, 0, 0], True):
                pass
            for _ in qkv_pair_steps(1):
                pass
            for _ in attention_steps(2, [2, 2, 0, 0], True):
                pass
            for _ in attention_steps(3, [0, 0, 0, 0], True, do_proj=True):
                pass

    nc.compile()
    return nc


def host_consts(t=T):
    """RoPE cos/sin (f32, matching the jax reference), causal big-mask, identity."""
    inv = (1.0 / (np.float32(10000.0) ** (np.arange(0, DH, 2, dtype=np.float32) / np.float32(DH)))).astype(np.float32)
    tt = np.arange(t, dtype=np.float32)
    fr = np.outer(tt, inv).astype(np.float32)       # [t, 64]
    emb = np.concatenate([fr, fr], axis=1)          # [t, 128]
    cosT = np.ascontiguousarray(np.cos(emb).T.astype(np.float32))
    sinTm = np.ascontiguousarray(np.sin(emb).T.astype(np.float32))
    jj = np.arange(128)[:, None]
    cc = np.arange(128)[None, :]
    bmask = (cc >= jj).astype(np.float32)
    ident = np.eye(128, dtype=np.float32)
    # signed half-rotation: (rotm.T @ x)[d] = -x[d+64] for d<64, x[d-64] else
    rotm = np.zeros((128, 128), dtype=np.float32)
    for d in range(64):
        rotm[d + 64, d] = -1.0
        rotm[d, d + 64] = 1.0
    return cosT, sinTm, bmask, ident, rotm


def _warrange(w):
    """[128*nh rows, D] head-major weight slice -> [128, nh*D] sbuf-ready layout:
    block h, col di*128+c of partition p  =  w[128*h + c, 128*di + p]."""
    nh = w.shape[0] // 128
    d = w.shape[1]
    out = np.empty((128, nh * d), dtype=np.float32)
    for h in range(nh):
        a = w[128 * h:128 * (h + 1), :].T.reshape(d // 128, 128, 128)  # [di, p, c]
        out[:, d * h:d * (h + 1)] = a.transpose(1, 0, 2).reshape(128, d)
    return out


def _wvarrange(w):
    """[512 rows, D] 4-head v-weights -> [128, 2*2*D]: per pair, di-major blocks of
    [even-head 128 cols | odd-head 128 cols]."""
    d = w.shape[1]
    blocks = []
    for p2 in range(2):
        e = w[256 * p2:256 * p2 + 128, :].T.reshape(d // 128, 128, 128)
        o = w[256 * p2 + 128:256 * p2 + 256, :].T.reshape(d // 128, 128, 128)
        pair = np.concatenate([e, o], axis=2)          # [di, p, 256]
        blocks.append(pair.transpose(1, 0, 2).reshape(128, 2 * d))
    return np.concatenate(blocks, axis=1)


def shard_inputs(x, w_qkv, w_proj, t=T, pv_dt="bfloat16"):
    """Build the 8 per-core input maps."""
    bdt = ml_dtypes.bfloat16 if pv_dt == "bfloat16" else np.float32
    cosT, sinTm, bmask, ident, rotm = host_consts(t)
    cosT = cosT.astype(bdt)
    sinTm = sinTm.astype(bdt)
    bmask = bmask.astype(bdt)
    ident = ident.astype(bdt)
    rotm = rotm.astype(bdt)
    d = x.shape[2]
    in_maps = []
    for c in range(8):
        b, g = divmod(c, 4)
        s0, s1 = 512 * g, 512 * (g + 1)
        in_maps.append(dict(
            xT=np.ascontiguousarray(x[b].T),
            wqh=_warrange(w_qkv[s0:s1, :]),
            wkh=_warrange(w_qkv[d + s0:d + s1, :]),
            wvh=_wvarrange(w_qkv[2 * d + s0:2 * d + s1, :]),
            wpT=np.ascontiguousarray(w_proj[:, s0:s1].T).astype(bdt),
            cosT=cosT, sinTm=sinTm, bmask=bmask, ident=ident, rotm=rotm,
        ))
    return in_maps


_NC_CACHE = {}


def get_nc(t=T, mm_dt="float32r", pv_dt="bfloat16"):
    key = (t, mm_dt, pv_dt)
    if key not in _NC_CACHE:
        _NC_CACHE[key] = build_nc(t=t, mm_dt=mm_dt, pv_dt=pv_dt)
    return _NC_CACHE[key]


def kernel(x, w_qkv, w_proj):
    x = np.asarray(x, dtype=np.float32)
    w_qkv = np.asarray(w_qkv, dtype=np.float32)
    w_proj = np.asarray(w_proj, dtype=np.float32)
    b_, t_, d_ = x.shape
    in_maps = shard_inputs(x, w_qkv, w_proj, t=t_)
    nc = get_nc(t=t_)
    res = run_bass_kernel_spmd(nc, in_maps, list(range(8))).results
    out = np.zeros((b_, t_, d_), dtype=np.float32)
    for c in range(8):
        b, _ = divmod(c, 4)
        out[b] += res[c]["y"]
    return out


# revision 59
# speedup vs baseline: 1.0271x; 1.0041x over previous
"""Trainium2 Bass kernel: causal self-attention with RoPE (B=2, T=2048, D=2048, H=16).

Sharding: 8 cores = 2-way data parallel over batch x 4-way tensor parallel over
heads.  Core c = 4*b + g computes batch b, heads 4g..4g+3, and produces a
partial output y_partial = attn_out[:, heads_g] @ w_proj[:, heads_g].T which the
host sums over g.

Per-core pipeline (v2 — tuned against the TimelineSim cost model):
  - qkv projection in fp32r (full-rate at free>=256), weights DMA'd in 512-col
    chunks interleaved with the x stream so the first matmul starts ~4us in.
  - q/k psums staged to SBUF by ScalarE copies (frees PSUM banks immediately;
    6-slot mm psum pool), RoPE on DVE reads the stage off the critical path.
  - exp on ScalarE writes p~ directly as bf16; PV matmul (free size 129) runs
    in bf16 at 1 cycle/row (fp32r would be 4x slower below 256 free).
  - o -> PE transpose in bf16 -> oT bf16 feeds the output projection (bf16
    weights) interleaved into head 3's attention, one t-range per i-block.
  - RoPE cos / sign-folded sin, causal 0/1 big-mask, identity are
    host-precomputed; consts stream in per-t-block chunks after the first
    weight chunks.
"""

import sys

import numpy as np
import ml_dtypes

for _p in ("/opt/trn_rl_repo", "/root/.axon_site/_ro/trn_rl_repo"):
    if _p not in sys.path:
        sys.path.append(_p)

import concourse.bass as bass
import concourse.bacc as bacc
import concourse.tile as tile
from concourse import mybir
from concourse.bass_utils import run_bass_kernel_spmd

F32 = mybir.dt.float32
BF16 = mybir.dt.bfloat16
AF = mybir.ActivationFunctionType

B, T, D, H = 2, 2048, 2048, 16
HPC = H // 4  # heads per core (4-way head TP)
DH = D // H   # 128
SCALE = float(DH) ** -0.5

TB = 512      # qkv-projection t-block (psum free width)
SB = 512      # attention i-block (score free width)


def build_nc(t=T, mm_dt="float32r", pv_dt="bfloat16"):
    """Build the SPMD per-core program.  `t` is the sequence length (smaller
    values are used for simulator validation)."""
    NT = t // 128    # token tiles
    TBE = min(TB, t)
    NTB = t // TBE   # qkv t-blocks
    sb = min(SB, t)
    NSB = t // sb    # attention i-blocks
    NIC = sb // 128  # i-chunks per i-block
    ND = D // 128    # contraction d-tiles
    C0 = sb - 128    # base column of the causal big-mask

    MDT = mybir.dt.float32r if (mm_dt == "float32r") else F32
    BDT = BF16 if (pv_dt == "bfloat16") else F32

    nc = bacc.Bacc("TRN2", target_bir_lowering=False, debug=False)

    xT = nc.dram_tensor("xT", [D, t], MDT, kind="ExternalInput").ap()
    wqh = nc.dram_tensor("wqh", [128, HPC * D], MDT, kind="ExternalInput").ap()
    wkh = nc.dram_tensor("wkh", [128, HPC * D], MDT, kind="ExternalInput").ap()
    wvh = nc.dram_tensor("wvh", [128, (HPC // 2) * 2 * D], MDT, kind="ExternalInput").ap()
    wpT = nc.dram_tensor("wpT", [HPC * DH, D], BDT, kind="ExternalInput").ap()
    cosT = nc.dram_tensor("cosT", [DH, t], BDT, kind="ExternalInput").ap()
    sinTm = nc.dram_tensor("sinTm", [DH, t], BDT, kind="ExternalInput").ap()
    bmask = nc.dram_tensor("bmask", [128, 128], BDT, kind="ExternalInput").ap()
    ident = nc.dram_tensor("ident", [128, 128], BDT, kind="ExternalInput").ap()
    rotm = nc.dram_tensor("rotm", [128, 128], BDT, kind="ExternalInput").ap()
    y = nc.dram_tensor("y", [t, D], F32, kind="ExternalOutput").ap()

    with tile.TileContext(nc) as tc:
        with (
            tc.tile_pool(name="consts", bufs=1) as cpool,
            tc.tile_pool(name="oTp", bufs=1) as opool,
            tc.tile_pool(name="qkp", bufs=2) as qkpool,
            tc.tile_pool(name="xtp", bufs=6) as xtp,
            tc.tile_pool(name="wqkp", bufs=1) as wqkp,
            tc.tile_pool(name="wvp", bufs=1) as wvp,
            tc.tile_pool(name="wpj", bufs=1) as wpj,
            tc.tile_pool(name="vep", bufs=2) as vep,
            tc.tile_pool(name="ptp", bufs=2 * NT + 2) as ptp,
            tc.tile_pool(name="tmpp", bufs=4) as tmpp,
            tc.tile_pool(name="smallp", bufs=6) as smallp,
            tc.tile_pool(name="ysp", bufs=6) as ysp,
            tc.tile_pool(name="ps_mm", bufs=6, space="PSUM") as ps_mm,
            tc.tile_pool(name="ps_sm", bufs=2, space="PSUM") as ps_sm,
        ):
            # const tiles allocated up front; DMAs issued later (chunked) so
            # the weight/x stream owns the head of the DMA queue.
            cos_sb = cpool.tile([DH, t], BDT, tag="cos")
            sin_sb = cpool.tile([DH, t], BDT, tag="sin")
            bm_sb = cpool.tile([128, 128], BDT, tag="bm")
            id_sb = cpool.tile([128, 128], BDT, tag="id")
            rot_sb = cpool.tile([128, 128], BDT, tag="rot")
            oT_sb = [
                opool.tile([DH, t], BDT, tag=f"oT{h}", name=f"oT{h}")
                for h in range(HPC)
            ]
            wp_sb = [
                wpj.tile([128, D], BDT, tag=f"wp{h}", name=f"wp{h}")
                for h in range(HPC)
            ]

            def rope_ip(qk, tb, name):
                """In-place RoPE on qk[:, tb-block] (holds the raw projection,
                parked there by a ScalarE psum copy).  The half-rotation (a
                cross-partition shuffle the DVE can't address) runs on the PE
                as a signed permutation matmul; the remaining DVE ops are all
                partition-aligned bf16 (2x mode)."""
                t0, t1_ = TBE * tb, TBE * (tb + 1)
                blk = qk[:, t0:t1_]
                rps = ps_mm.tile([128, TBE], F32, tag="mm", name=f"rot_{name}")
                nc.tensor.matmul(rps[:], rot_sb[:], blk, start=True, stop=True)
                r1 = tmpp.tile([128, TBE], BDT, tag="r1", name=f"r1_{name}")
                nc.vector.tensor_mul(r1[:], blk, cos_sb[:, t0:t1_])
                r2 = tmpp.tile([128, TBE], BDT, tag="r2", name=f"r2_{name}")
                nc.vector.tensor_mul(r2[:], rps[:], sin_sb[:, t0:t1_])
                nc.vector.tensor_add(blk, r1[:], r2[:])

            def emit_proj(tt, on_act=False):
                """y[128*tt:128*(tt+1), :] = sum_h oT_h[:, tt].T @ wp_h.
                on_act routes the psum drain to ScalarE (used for the blocks
                issued ahead of B chunks, whose DVE normalizes must not queue
                behind these copies)."""
                for db in range(D // 512):
                    ps = ps_mm.tile([128, 512], F32, tag="mm", name=f"psy{tt}_{db}")
                    for h in range(HPC):
                        nc.tensor.matmul(
                            ps[:],
                            oT_sb[h][:, 128 * tt:128 * (tt + 1)],
                            wp_sb[h][:, 512 * db:512 * (db + 1)],
                            start=(h == 0), stop=(h == HPC - 1))
                    yst = ysp.tile([128, 512], F32, tag="yst", name=f"yst{tt}_{db}")
                    if on_act:
                        nc.scalar.copy(yst[:], ps[:])
                    else:
                        nc.vector.tensor_copy(yst[:], ps[:])
                    nc.sync.dma_start(
                        y[128 * tt:128 * (tt + 1), 512 * db:512 * (db + 1)], yst[:])

            pair_bufs = {}
            pend_ropes = []

            def qkv_pair_steps(p2):
                """Generator: pair-p2 weight DMAs + qkv projection; yields
                once per t-block so attention chunks of the previous pair can
                interleave into the PE stream."""
                h = 2 * p2
                q_sbs = [None, None]
                k_sbs = [None, None]
                vext = [None, None]
                wq0 = wqkp.tile([128, D], MDT, tag="wq0", name=f"wq0_{p2}")
                wk0 = wqkp.tile([128, D], MDT, tag="wk0", name=f"wk0_{p2}")
                wq1 = wqkp.tile([128, D], MDT, tag="wq1", name=f"wq1_{p2}")
                wk1 = wqkp.tile([128, D], MDT, tag="wk1", name=f"wk1_{p2}")
                wv_sb = wvp.tile([128, 2 * D], MDT, tag="wv", name=f"wv{p2}")
                vext[0] = vep.tile([128, NT * 129], BDT, tag="ve0", name=f"ve0_{p2}")
                vext[1] = vep.tile([128, NT * 129], BDT, tag="ve1", name=f"ve1_{p2}")
                nc.vector.memset(vext[0][:], 1.0)
                nc.vector.memset(vext[1][:], 1.0)
                q_sbs[0] = qkpool.tile([DH, t], BDT, tag="q0", name=f"q0_{p2}")
                k_sbs[0] = qkpool.tile([DH, t], BDT, tag="k0", name=f"k0_{p2}")
                q_sbs[1] = qkpool.tile([DH, t], BDT, tag="q1", name=f"q1_{p2}")
                k_sbs[1] = qkpool.tile([DH, t], BDT, tag="k1", name=f"k1_{p2}")
                pair_bufs[p2] = {"q": q_sbs, "k": k_sbs, "ve": vext}

                if p2 == 1:
                    # prefetch the projection weights during pair-1 qkv
                    for hh in range(HPC):
                        nc.sync.dma_start(wp_sb[hh][:], wpT[128 * hh:128 * (hh + 1), :])

                # ---- qkv projection for the pair (one xT pass) ----
                ttpb = TBE // 128
                nvp = (ttpb + 1) // 2
                for tb in range(NTB):
                    t0, t1_ = TBE * tb, TBE * (tb + 1)
                    ps_q0 = ps_mm.tile([128, TBE], F32, tag="mm", name=f"psq0_{p2}_{tb}")
                    ps_k0 = ps_mm.tile([128, TBE], F32, tag="mm", name=f"psk0_{p2}_{tb}")
                    ps_q1 = ps_mm.tile([128, TBE], F32, tag="mm", name=f"psq1_{p2}_{tb}")
                    ps_k1 = ps_mm.tile([128, TBE], F32, tag="mm", name=f"psk1_{p2}_{tb}")
                    ps_vs = [
                        ps_sm.tile([128, 512], F32, tag="sm", name=f"psv{p2}_{tb}_{i}")
                        for i in range(nvp)
                    ]
                    for di in range(ND):
                        d0, d1 = 128 * di, 128 * (di + 1)
                        # weight chunks interleaved with the x stream so the
                        # first matmuls start after ~0.5MB of DMA
                        wq_chunks = []
                        if tb == 0 and di % 4 == 0:
                            wq_chunks = [(512 * (di // 4), 512 * (di // 4 + 1))]
                        for c0_, c1_ in wq_chunks:
                            nc.sync.dma_start(wq0[:, c0_:c1_], wqh[:, D * h + c0_:D * h + c1_])
                        xt_t = xtp.tile([128, TBE], MDT, tag="xt", name=f"xt{p2}_{tb}_{di}")
                        nc.sync.dma_start(xt_t[:], xT[d0:d1, t0:t1_])
                        for c0_, c1_ in wq_chunks:
                            nc.sync.dma_start(wk0[:, c0_:c1_], wkh[:, D * h + c0_:D * h + c1_])
                            nc.sync.dma_start(wq1[:, c0_:c1_], wqh[:, D * (h + 1) + c0_:D * (h + 1) + c1_])
                            nc.sync.dma_start(wk1[:, c0_:c1_], wkh[:, D * (h + 1) + c0_:D * (h + 1) + c1_])
                        if tb == 0 and di % 2 == 0:
                            v0, v1 = 512 * (di // 2), 512 * (di // 2 + 1)
                            nc.sync.dma_start(wv_sb[:, v0:v1], wvh[:, 2 * D * p2 + v0:2 * D * p2 + v1])
                        st, sp = di == 0, di == ND - 1
                        nc.tensor.matmul(ps_q0[:], wq0[:, d0:d1], xt_t[:], start=st, stop=sp)
                        nc.tensor.matmul(ps_k0[:], wk0[:, d0:d1], xt_t[:], start=st, stop=sp)
                        nc.tensor.matmul(ps_q1[:], wq1[:, d0:d1], xt_t[:], start=st, stop=sp)
                        nc.tensor.matmul(ps_k1[:], wk1[:, d0:d1], xt_t[:], start=st, stop=sp)
                        for tt in range(ttpb):
                            # start=True clears the whole bank: only the
                            # even half may set it; the odd half's first
                            # write lands via has_written=0 overwrite.
                            nc.tensor.matmul(
                                ps_vs[tt // 2][:, 256 * (tt % 2):256 * (tt % 2) + 256],
                                xt_t[:, 128 * tt:128 * (tt + 1)],
                                wv_sb[:, 256 * di:256 * (di + 1)],
                                start=(st and tt % 2 == 0), stop=sp,
                                skip_group_check=True)
                    if p2 == 0:
                        # stream the consts the tb's ropes need right
                        # behind its x tiles
                        nc.sync.dma_start(cos_sb[:, t0:t1_], cosT[:, t0:t1_])
                        nc.sync.dma_start(sin_sb[:, t0:t1_], sinTm[:, t0:t1_])
                        if tb == 0:
                            nc.sync.dma_start(rot_sb[:], rotm[:])
                        if tb == 1:
                            nc.sync.dma_start(bm_sb[:], bmask[:])
                            nc.sync.dma_start(id_sb[:], ident[:])
                    # park raw psums straight into the q/k tiles on
                    # ScalarE (frees the banks without waiting on DVE);
                    # RoPE rotates in place later.  At pair 0's last t-block
                    # the first attention chunks are waiting on these slots,
                    # so split the copies across ScalarE and DVE.
                    split = tb == NTB - 1
                    for i_, (ps, dst) in enumerate(((ps_q0, q_sbs[0]), (ps_k0, k_sbs[0]), (ps_q1, q_sbs[1]), (ps_k1, k_sbs[1]))):
                        if split and i_ % 2 == 1:
                            nc.vector.tensor_copy(dst[:, t0:t1_], ps[:])
                        else:
                            nc.scalar.copy(dst[:, t0:t1_], ps[:])
                    vcopy = nc.vector.tensor_copy if split else nc.scalar.copy
                    for tt in range(ttpb):
                        gt = tb * ttpb + tt
                        o0 = 256 * (tt % 2)
                        vcopy(
                            vext[0][:, 129 * gt:129 * gt + 128],
                            ps_vs[tt // 2][:, o0:o0 + 128])
                        vcopy(
                            vext[1][:, 129 * gt:129 * gt + 128],
                            ps_vs[tt // 2][:, o0 + 128:o0 + 256])
                    # RoPE policy: pair 0 defers q1/k1 + the last tb's q0/k0
                    # into the attention stream (so the pair-boundary DVE
                    # queue starts with the diagonal masks).  Pair 1's tb
                    # loop is already interleaved with h0/h1 attention, so
                    # in-loop ropes hide in the t-block gaps; only the last
                    # tb's four ropes defer (consumed by h2's chunks).
                    last = tb == NTB - 1
                    if not last:
                        rope_ip(q_sbs[0], tb, f"q0_{p2}_{tb}")
                        rope_ip(k_sbs[0], tb, f"k0_{p2}_{tb}")
                        if p2 == 1:
                            rope_ip(q_sbs[1], tb, f"q1_{p2}_{tb}")
                            rope_ip(k_sbs[1], tb, f"k1_{p2}_{tb}")
                    else:
                        pend_ropes.append((q_sbs[0], tb, f"q0_{p2}_{tb}"))
                        pend_ropes.append((k_sbs[0], tb, f"k0_{p2}_{tb}"))
                        pend_ropes.append((q_sbs[1], tb, f"q1_{p2}_{tb}"))
                        pend_ropes.append((k_sbs[1], tb, f"k1_{p2}_{tb}"))
                    if p2 == 0 and not last:
                        pend_ropes.append((q_sbs[1], tb, f"q1_{p2}_{tb}"))
                        pend_ropes.append((k_sbs[1], tb, f"k1_{p2}_{tb}"))
                    yield

            def attention_steps(h, quotas, pipelined, do_proj=False):
                """Generator for head h's attention, yielding once per chunk.
                pipelined=True: A0 A1 B0 A2 B1 A3 B2 B3 (B lags A by one).
                pipelined=False: A0 B0 A1 B1 ... (B right after its A).
                quotas[i] deferred ropes are issued after the i-th B chunk."""
                par = h % 2
                bufs = pair_bufs[h // 2]
                q_sb, k_sb = bufs["q"][par], bufs["k"][par]
                ve = bufs["ve"][par]
                if par == 0 and pend_ropes:
                    # rotate this head's q/k first (its own ib3 needs them)
                    mine = [e for e in pend_ropes if e[0] is q_sb or e[0] is k_sb]
                    rest = [e for e in pend_ropes if not (e[0] is q_sb or e[0] is k_sb)]
                    pend_ropes[:] = mine + rest

                def stage_a(ib):
                    """scores + exp + diagonal mask for i-block ib."""
                    i0 = sb * ib
                    jt_max = (i0 + sb) // 128 - 1  # inclusive
                    pts = [None] * (jt_max + 1)
                    for jt in range(jt_max + 1):
                        s_ps = ps_mm.tile([128, sb], F32, tag="mm", name=f"s{h}_{ib}_{jt}")
                        nc.tensor.matmul(
                            s_ps[:],
                            k_sb[:, 128 * jt:128 * (jt + 1)],
                            q_sb[:, i0:i0 + sb],
                            start=True, stop=True)
                        pt_t = ptp.tile([128, sb], BDT, tag="pt", name=f"pt{h}_{ib}_{jt}")
                        nc.scalar.activation(pt_t[:], s_ps[:], AF.Exp, scale=SCALE)
                        m = jt - NIC * ib
                        if m >= 0:
                            # only the 128x128 sub-block straddling the causal
                            # diagonal needs masking: fully-masked sub-blocks
                            # are never read by the PV loop bounds in stage_b.
                            # Runs on the otherwise-idle GpSimd engine so it
                            # never queues behind DVE ropes/normalizes.
                            pm = pt_t[:, 128 * m:128 * (m + 1)]
                            nc.gpsimd.tensor_mul(pm, pm, bm_sb[:])
                        pts[jt] = pt_t
                    return pts

                def stage_b(ib, pts, bi):
                    """PV + normalize + transpose for i-block ib."""
                    i0 = sb * ib

                    def finish(ic, pv):
                        rc = smallp.tile([128, 1], F32, tag="rc", name=f"rc{h}_{ib}_{ic}")
                        nc.vector.reciprocal(rc[:], pv[:, 128:129])
                        o_sb = smallp.tile([128, 128], BDT, tag="o", name=f"o{h}_{ib}_{ic}")
                        nc.vector.tensor_scalar_mul(o_sb[:], pv[:, 0:128], rc[:])
                        ot_ps = ps_mm.tile([128, 128], BDT, tag="mm", name=f"otp{h}_{ib}_{ic}")
                        nc.tensor.transpose(ot_ps[:], o_sb[:], id_sb[:])
                        c0 = i0 + 128 * ic
                        nc.vector.tensor_copy(oT_sb[h][:, c0:c0 + 128], ot_ps[:])

                    prev = None
                    for ic in range(NIC):
                        last_jt = NIC * ib + ic
                        pv = ps_sm.tile([128, 129], F32, tag="sm", name=f"pv{h}_{ib}_{ic}")
                        for jt in range(last_jt + 1):
                            nc.tensor.matmul(
                                pv[:],
                                pts[jt][:, 128 * ic:128 * (ic + 1)],
                                ve[:, 129 * jt:129 * (jt + 1)],
                                start=(jt == 0), stop=(jt == last_jt))
                        if prev is not None:
                            finish(*prev)
                        prev = (ic, pv)
                    finish(*prev)
                    if do_proj:
                        # head 3's oT columns complete per i-block: fold the
                        # output projection for those t-tiles in right away
                        # (the last i-block drains on both engines so the
                        # kernel tail is DMA-limited, not copy-limited)
                        for tt in range(NIC * ib, NIC * (ib + 1)):
                            emit_proj(tt, on_act=(ib == NSB - 1 and tt % 2 == 0))
                    # deferred RoPEs enter the DVE queue after the B chunk so
                    # the masks and PV-psum-freeing normalizes stay ahead
                    consume_ropes(quotas[bi])

                if pipelined:
                    pts_prev = None
                    for ib in range(NSB):
                        pts_cur = stage_a(ib)
                        yield
                        if pts_prev is not None:
                            stage_b(ib - 1, pts_prev, ib - 1)
                            yield
                        pts_prev = pts_cur
                    stage_b(NSB - 1, pts_prev, NSB - 1)
                    yield
                else:
                    for ib in range(NSB):
                        pts = stage_a(ib)
                        yield
                        stage_b(ib, pts, ib)
                        yield

            def stepn(g, n):
                for _ in range(n):
                    next(g)

            def consume_ropes(n):
                for qk, tb_, nm_ in pend_ropes[:n]:
                    rope_ip(qk, tb_, nm_)
                del pend_ropes[:n]

            # ---- schedule -------------------------------------------
            # pair 0 qkv alone (nothing to overlap), then h0/h1 attention
            # chunks interleaved into pair 1's t-blocks (attention is
            # exp-bound on ScalarE; the qkv matmuls keep PE busy), then
            # h2/h3 attention interleaved with the output projection.
            for _ in qkv_pair_steps(0):
                pass
            for _ in attention_steps(0, [3, 3, 2, 2], True):
                pass
            for _ in attention_steps(1, [0, 0, 0, 0], True):
                pass
            for _ in qkv_pair_steps(1):
                pass
            for _ in attention_steps(2, [2, 2, 0, 0], True):
                pass
            for _ in attention_steps(3, [0, 0, 0, 0], True, do_proj=True):
                pass

    nc.compile()
    return nc


def host_consts(t=T):
    """RoPE cos/sin (f32, matching the jax reference), causal big-mask, identity."""
    inv = (1.0 / (np.float32(10000.0) ** (np.arange(0, DH, 2, dtype=np.float32) / np.float32(DH)))).astype(np.float32)
    tt = np.arange(t, dtype=np.float32)
    fr = np.outer(tt, inv).astype(np.float32)       # [t, 64]
    emb = np.concatenate([fr, fr], axis=1)          # [t, 128]
    cosT = np.ascontiguousarray(np.cos(emb).T.astype(np.float32))
    sinTm = np.ascontiguousarray(np.sin(emb).T.astype(np.float32))
    jj = np.arange(128)[:, None]
    cc = np.arange(128)[None, :]
    bmask = (cc >= jj).astype(np.float32)
    ident = np.eye(128, dtype=np.float32)
    # signed half-rotation: (rotm.T @ x)[d] = -x[d+64] for d<64, x[d-64] else
    rotm = np.zeros((128, 128), dtype=np.float32)
    for d in range(64):
        rotm[d + 64, d] = -1.0
        rotm[d, d + 64] = 1.0
    return cosT, sinTm, bmask, ident, rotm


def _warrange(w):
    """[128*nh rows, D] head-major weight slice -> [128, nh*D] sbuf-ready layout:
    block h, col di*128+c of partition p  =  w[128*h + c, 128*di + p]."""
    nh = w.shape[0] // 128
    d = w.shape[1]
    out = np.empty((128, nh * d), dtype=np.float32)
    for h in range(nh):
        a = w[128 * h:128 * (h + 1), :].T.reshape(d // 128, 128, 128)  # [di, p, c]
        out[:, d * h:d * (h + 1)] = a.transpose(1, 0, 2).reshape(128, d)
    return out


def _wvarrange(w):
    """[512 rows, D] 4-head v-weights -> [128, 2*2*D]: per pair, di-major blocks of
    [even-head 128 cols | odd-head 128 cols]."""
    d = w.shape[1]
    blocks = []
    for p2 in range(2):
        e = w[256 * p2:256 * p2 + 128, :].T.reshape(d // 128, 128, 128)
        o = w[256 * p2 + 128:256 * p2 + 256, :].T.reshape(d // 128, 128, 128)
        pair = np.concatenate([e, o], axis=2)          # [di, p, 256]
        blocks.append(pair.transpose(1, 0, 2).reshape(128, 2 * d))
    return np.concatenate(blocks, axis=1)


def shard_inputs(x, w_qkv, w_proj, t=T, pv_dt="bfloat16"):
    """Build the 8 per-core input maps."""
    bdt = ml_dtypes.bfloat16 if pv_dt == "bfloat16" else np.float32
    cosT, sinTm, bmask, ident, rotm = host_consts(t)
    cosT = cosT.astype(bdt)
    sinTm = sinTm.astype(bdt)
    bmask = bmask.astype(bdt)
    ident = ident.astype(bdt)
    rotm = rotm.astype(bdt)
    d = x.shape[2]
    in_maps = []
    for c in range(8):
        b, g = divmod(c, 4)
        s0, s1 = 512 * g, 512 * (g + 1)
        in_maps.append(dict(
            xT=np.ascontiguousarray(x[b].T),
            wqh=_warrange(w_qkv[s0:s1, :]),
            wkh=_warrange(w_qkv[d + s0:d + s1, :]),
            wvh=_wvarrange(w_qkv[2 * d + s0:2 * d + s1, :]),
            wpT=np.ascontiguousarray(w_proj[:, s0:s1].T).astype(bdt),
            cosT=cosT, sinTm=sinTm, bmask=bmask, ident=ident, rotm=rotm,
        ))
    return in_maps


_NC_CACHE = {}


def get_nc(t=T, mm_dt="float32r", pv_dt="bfloat16"):
    key = (t, mm_dt, pv_dt)
    if key not in _NC_CACHE:
        _NC_CACHE[key] = build_nc(t=t, mm_dt=mm_dt, pv_dt=pv_dt)
    return _NC_CACHE[key]


def kernel(x, w_qkv, w_proj):
    x = np.asarray(x, dtype=np.float32)
    w_qkv = np.asarray(w_qkv, dtype=np.float32)
    w_proj = np.asarray(w_proj, dtype=np.float32)
    b_, t_, d_ = x.shape
    in_maps = shard_inputs(x, w_qkv, w_proj, t=t_)
    nc = get_nc(t=t_)
    res = run_bass_kernel_spmd(nc, in_maps, list(range(8))).results
    out = np.zeros((b_, t_, d_), dtype=np.float32)
    for c in range(8):
        b, _ = divmod(c, 4)
        out[b] += res[c]["y"]
    return out
